# revision 17
# baseline (speedup 1.0000x reference)
"""GatedLinearRecurrence Trainium2 kernel (8-core SPMD, Bass/Tile).

Strategy: shard (batch=2) x (4 sequence chunks of 1024 tokens) across 8 cores.
Each core processes 1152 tokens: a 128-token warm-up window (re-computed
redundantly; the recurrence decay e^{-~100} makes carry-in truncation error
~1e-24) followed by its 1024 "main" tokens.  No collectives needed.

Per-core pipeline (channels-on-partitions, tokens-on-free layout):
  LN(x) [t,d] -> PE-transpose -> x̂T [d,t] -> in_proj (f32r matmul)
  -> causal depthwise conv (4 shifted tensor_scalar ops) -> silu -> mask
  -> gate matmul -> sigmoid -> b=-(1-a)*xc -> tensor_tensor_scan (h=-h)
  -> y*silu(z) -> out_proj -> residual subtract -> out [t,dm].

The sign trick: scan data1 = (a-1)*x_conv = -b gives -h; -h*silu(z) = -yg;
out = x - matmul(-yg) = x + proj(yg).

Scheduling notes: x̂T is stored in 384-column chunk tiles so in_proj can
start before layernorm finishes; PSUM evacuations ride the Scalar engine
(Identity/Copy activations) to unload DVE; the scan is chunked (chained via
`initial=`) and interleaved into the gate loop so out_proj follows with no
PE gap; scan outputs reuse the a-tile pool slots (same tag) to fit SBUF.
"""
import sys

for p in ("/opt/trn_rl_repo", "/root/.axon_site/_ro/trn_rl_repo"):
    if p not in sys.path:
        sys.path.insert(0, p)

import numpy as np

import concourse.bass as bass
import concourse.bacc as bacc
import concourse.tile as tile
import concourse.mybir as mybir
from concourse.bass_utils import run_bass_kernel_spmd
from concourse.masks import make_identity

F32 = mybir.dt.float32
F32R = mybir.dt.float32r
AF = mybir.ActivationFunctionType
OP = mybir.AluOpType

B, L, D = 2, 4096, 1024
DI = 2048            # d_inner
NT = 1152            # tokens per core (128 warm-up + 1024 main)
W = 128              # warm-up tokens
CHUNK = 1024
NTT = NT // 128      # 9 token tiles
KD = D // 128        # 8 k-tiles over d_model
KC = DI // 128       # 16 k-tiles over d_inner
TC = 384             # matmul N chunk (3 per core)
NTC = NT // TC
EPS = 1e-5

_cache = {}


def _build():
    nc = bacc.Bacc(None, target_bir_lowering=False)

    x_h = nc.dram_tensor("x", [NT, D], F32, kind="ExternalInput")
    w1x_h = nc.dram_tensor("w1x", [D, DI], F32, kind="ExternalInput")
    w1z_h = nc.dram_tensor("w1z", [D, DI], F32, kind="ExternalInput")
    gw_h = nc.dram_tensor("gw", [DI, DI], F32, kind="ExternalInput")
    op_h = nc.dram_tensor("opw", [DI, D], F32, kind="ExternalInput")
    convw_h = nc.dram_tensor("convw", [128, KC * 4], F32, kind="ExternalInput")
    convb_h = nc.dram_tensor("convb", [128, KC], F32, kind="ExternalInput")
    gateb_h = nc.dram_tensor("gateb", [128, KC], F32, kind="ExternalInput")
    normb_h = nc.dram_tensor("normb", [128, KD], F32, kind="ExternalInput")
    mask_h = nc.dram_tensor("mask", [1, NT], F32, kind="ExternalInput")
    out_h = nc.dram_tensor("out", [CHUNK, D], F32, kind="ExternalOutput")
    z_h = nc.dram_tensor("z_scratch", [KC, 128, NT], F32, kind="Internal")
    yg_h = nc.dram_tensor("yg_scratch", [KC, 128, CHUNK], F32, kind="Internal")

    with tile.TileContext(nc) as tc:
        with tc.tile_pool(name="consts", bufs=1) as consts, \
             tc.tile_pool(name="ay", bufs=4) as ayp, \
             tc.tile_pool(name="psmm", bufs=4, space="PSUM") as psmm:

            ident = consts.tile([128, 128], F32, name="ident")
            make_identity(nc, ident)
            mask_sb = consts.tile([128, W], F32R, name="mask_sb")
            nc.gpsimd.dma_start(
                out=mask_sb,
                in_=bass.AP(tensor=mask_h, offset=0, ap=[[0, 128], [1, W]]).bitcast(F32R),
            )
            convw = consts.tile([128, KC * 4], F32, name="convw")
            nc.gpsimd.dma_start(out=convw, in_=convw_h.ap())
            convb = consts.tile([128, KC], F32, name="convb")
            nc.gpsimd.dma_start(out=convb, in_=convb_h.ap())
            gateb = consts.tile([128, KC], F32, name="gateb")
            nc.gpsimd.dma_start(out=gateb, in_=gateb_h.ap())
            normb = consts.tile([128, KD], F32, name="normb")
            nc.gpsimd.dma_start(out=normb, in_=normb_h.ap())
            eps_t = consts.tile([128, 1], F32, name="eps_t")
            nc.vector.memset(eps_t, EPS)

            with tc.tile_pool(name="xcp", bufs=1) as xcp:
                xc = [xcp.tile([128, NT], F32R, name=f"xct{e}") for e in range(KC)]

                # ---- S1-S3: LN, transpose, in_proj (x & z), conv, silu ----
                with tc.tile_pool(name="xT", bufs=1) as xTp, \
                     tc.tile_pool(name="s1roll", bufs=2) as s1r, \
                     tc.tile_pool(name="stat", bufs=3) as stp, \
                     tc.tile_pool(name="w1s", bufs=3) as ws, \
                     tc.tile_pool(name="zr", bufs=3) as zrp, \
                     tc.tile_pool(name="pstr", bufs=2, space="PSUM") as pstr:

                    # x̂T chunk tiles [d-tile][t-chunk] -> finer deps: the
                    # first in_proj matmuls start after 3 LN iterations.
                    xT = [[xTp.tile([128, TC], F32R, name=f"xTt{d_}_{c_}")
                           for c_ in range(NTC)] for d_ in range(KD)]

                    for it in range(NTT):
                        tc3, col = it // 3, (it % 3) * 128
                        xt = s1r.tile([128, D], F32, tag="xt", name="xt")
                        nc.sync.dma_start(out=xt, in_=x_h.ap()[it * 128:(it + 1) * 128, :])
                        stats = stp.tile([128, 2, 6], F32, tag="stats", name="stats")
                        nc.vector.bn_stats(out=stats[:, 0, :], in_=xt[:, 0:512])
                        nc.vector.bn_stats(out=stats[:, 1, :], in_=xt[:, 512:1024])
                        mv = stp.tile([128, 2], F32, tag="mv", name="mv")
                        nc.vector.bn_aggr(out=mv, in_=stats)
                        rstd = stp.tile([128, 1], F32, tag="rstd", name="rstd")
                        nc.scalar.activation(out=rstd, in_=mv[:, 1:2], func=AF.Sqrt,
                                             bias=eps_t, scale=1.0)
                        nc.vector.reciprocal(out=rstd, in_=rstd)
                        nc.vector.tensor_scalar(out=xt, in0=xt, scalar1=mv[:, 0:1],
                                                scalar2=rstd, op0=OP.subtract, op1=OP.mult)
                        for d_ in range(KD):
                            pst = pstr.tile([128, 128], F32, tag="tr", name="pst")
                            nc.tensor.transpose(pst, xt[:, d_ * 128:(d_ + 1) * 128], ident)
                            # evac + norm_b on the Scalar engine
                            nc.scalar.activation(
                                out=xT[d_][tc3][:, col:col + 128], in_=pst,
                                func=AF.Identity, bias=normb[:, d_:d_ + 1], scale=1.0)

                    # in_proj x-half + conv + silu + warm-up mask.
                    # Two 8-et weight groups, tc3-outer inside the group: the
                    # tc3=0 chains of 8 ets fill the PE while layernorm is
                    # still producing the later x̂T chunks.  Conv runs
                    # per-chunk with a 3-column halo carried between chunks.
                    halo = stp.tile([128, KC, 4], F32, tag="halo", bufs=1,
                                    name="halo")
                    wts = {}
                    for g in range(2):
                        ets = range(g * 8, g * 8 + 8)
                        for tc3 in range(NTC):
                            for et in ets:
                                if tc3 == 0:
                                    wt = ws.tile([128, KD, 128], F32R, tag="w1",
                                                 bufs=9, name=f"wt{et}")
                                    nc.sync.dma_start(
                                        out=wt,
                                        in_=w1x_h.ap()[:, et * 128:(et + 1) * 128]
                                        .rearrange("(kt p) e -> p kt e", p=128)
                                        .bitcast(F32R))
                                    wts[et] = wt
                                ps = psmm.tile([128, TC], F32, tag="mm", name="ps")
                                for kt in range(KD):
                                    nc.tensor.matmul(
                                        ps, wts[et][:, kt, :], xT[kt][tc3],
                                        start=(kt == 0), stop=(kt == KD - 1))
                                xin = s1r.tile([128, TC + 3], F32, tag="xin",
                                               bufs=3, name="xin")
                                if tc3 == 0:
                                    nc.vector.memset(xin[:, 0:3], 0.0)
                                else:
                                    nc.vector.tensor_copy(
                                        xin[:, 0:3], halo[:, et, 0:3])
                                nc.scalar.copy(out=xin[:, 3:3 + TC], in_=ps)
                                if tc3 < NTC - 1:
                                    nc.vector.tensor_copy(
                                        halo[:, et, 0:3], xin[:, TC:TC + 3])
                                tmp = s1r.tile([128, TC], F32, tag="ctmp",
                                               bufs=3, name="ctmp")
                                nc.vector.tensor_scalar_mul(
                                    tmp, xin[:, 0:TC], convw[:, et * 4:et * 4 + 1])
                                for k in range(1, 4):
                                    nc.vector.scalar_tensor_tensor(
                                        out=tmp, in0=xin[:, k:k + TC],
                                        scalar=convw[:, et * 4 + k:et * 4 + k + 1],
                                        in1=tmp, op0=OP.mult, op1=OP.add)
                                nc.scalar.activation(
                                    out=xc[et][:, tc3 * TC:(tc3 + 1) * TC], in_=tmp,
                                    func=AF.Silu, bias=convb[:, et:et + 1], scale=1.0)
                                if tc3 == 0:
                                    nc.vector.tensor_mul(
                                        xc[et][:, 0:W], xc[et][:, 0:W], mask_sb)

                    # in_proj z-half + silu -> HBM scratch
                    for et in range(KC):
                        wt = ws.tile([128, KD, 128], F32R, tag="w1", bufs=9, name="wtz")
                        nc.sync.dma_start(
                            out=wt,
                            in_=w1z_h.ap()[:, et * 128:(et + 1) * 128]
                            .rearrange("(kt p) e -> p kt e", p=128).bitcast(F32R))
                        for tc3 in range(NTC):
                            ps = psmm.tile([128, TC], F32, tag="mm", name="psz")
                            for kt in range(KD):
                                nc.tensor.matmul(
                                    ps, wt[:, kt, :], xT[kt][tc3],
                                    start=(kt == 0), stop=(kt == KD - 1))
                            zroll = zrp.tile([128, TC], F32, tag="zr", name="zroll")
                            nc.scalar.activation(out=zroll, in_=ps, func=AF.Silu)
                            nc.sync.dma_start(
                                out=z_h.ap()[et, :, tc3 * TC:(tc3 + 1) * TC], in_=zroll)

                # out_proj weights load early (no deps) on the GP queue; the
                # pool opens after the S1-S3 pools close so the space exists.
                with tc.tile_pool(name="opw", bufs=1) as opp:
                    opt = [opp.tile([128, D], F32R, name=f"opt{k}") for k in range(KC)]
                    for kt in range(KC):
                        nc.gpsimd.dma_start(
                            out=opt[kt],
                            in_=op_h.ap()[kt * 128:(kt + 1) * 128, :].bitcast(F32R))

                    # ---- S4-S6: gate matmul, sigmoid, chunked scan, y*silu(z) ----
                    with tc.tile_pool(name="gws", bufs=2) as gs, \
                         tc.tile_pool(name="s6roll", bufs=3) as s6r:

                        for et in range(KC):
                            gt = gs.tile([128, KC, 128], F32R, tag="gw", name="gt")
                            nc.gpsimd.dma_start(
                                out=gt,
                                in_=gw_h.ap()[:, et * 128:(et + 1) * 128]
                                .rearrange("(kt p) e -> p kt e", p=128).bitcast(F32R))
                            zl = s6r.tile([128, CHUNK], F32, tag="zl", name="zl")
                            nc.gpsimd.dma_start(out=zl, in_=z_h.ap()[et, :, W:NT])
                            a_t = ayp.tile([128, NT], F32R, tag="ay", name=f"at{et}")
                            for tc3 in range(NTC):
                                ps = psmm.tile([128, TC], F32, tag="mm", name="psg")
                                for kt in range(KC):
                                    nc.tensor.matmul(
                                        ps, gt[:, kt, :], xc[kt][:, tc3 * TC:(tc3 + 1) * TC],
                                        start=(kt == 0), stop=(kt == KC - 1))
                                nc.scalar.activation(
                                    out=a_t[:, tc3 * TC:(tc3 + 1) * TC], in_=ps,
                                    func=AF.Sigmoid, bias=gateb[:, et:et + 1], scale=1.0)
                            # chunked scan chained via initial=, interleaved with
                            # the gate matmuls of later et (xc stays intact);
                            # -yg spills to HBM and is re-gathered by out_proj.
                            y_t = s6r.tile([128, NT], F32, tag="y", name=f"yt{et}")
                            for tc3 in range(NTC):
                                bt = s6r.tile([128, TC], F32, tag="bt", name="bt")
                                nc.vector.scalar_tensor_tensor(
                                    out=bt, in0=a_t[:, tc3 * TC:(tc3 + 1) * TC], scalar=1.0,
                                    in1=xc[et][:, tc3 * TC:(tc3 + 1) * TC],
                                    op0=OP.subtract, op1=OP.mult)
                                init = 0.0 if tc3 == 0 else y_t[:, tc3 * TC - 1:tc3 * TC]
                                nc.vector.tensor_tensor_scan(
                                    out=y_t[:, tc3 * TC:(tc3 + 1) * TC],
                                    data0=a_t[:, tc3 * TC:(tc3 + 1) * TC],
                                    data1=bt, initial=init, op0=OP.mult, op1=OP.add)
                            nc.vector.tensor_mul(y_t[:, W:NT], y_t[:, W:NT], zl)  # -yg
                            nc.sync.dma_start(out=yg_h.ap()[et], in_=y_t[:, W:NT])

                    # ---- S7: out_proj + residual ----
                    with tc.tile_pool(name="s7roll", bufs=3) as s7r, \
                         tc.tile_pool(name="psop", bufs=4, space="PSUM") as psop:
                        for tb in range(CHUNK // 128):
                            ygts = []
                            for kt in range(KC):
                                ygt = s7r.tile([128, 128], F32R, tag="ygt",
                                               bufs=24, name=f"ygt{kt}")
                                nc.gpsimd.dma_start(
                                    out=ygt,
                                    in_=yg_h.ap()[kt, :, tb * 128:(tb + 1) * 128]
                                    .bitcast(F32R))
                                ygts.append(ygt)
                            xres = s7r.tile([128, D], F32, tag="xres", name="xres")
                            nc.sync.dma_start(
                                out=xres,
                                in_=x_h.ap()[W + tb * 128:W + (tb + 1) * 128, :])
                            outt = s7r.tile([128, D], F32, tag="outt", name="outt")
                            for nb in range(2):
                                ps = psop.tile([128, 512], F32, tag="op", name="pso")
                                for kt in range(KC):
                                    nc.tensor.matmul(
                                        ps, ygts[kt],
                                        opt[kt][:, nb * 512:(nb + 1) * 512],
                                        start=(kt == 0), stop=(kt == KC - 1))
                                nc.vector.tensor_sub(
                                    outt[:, nb * 512:(nb + 1) * 512],
                                    xres[:, nb * 512:(nb + 1) * 512], ps)
                            nc.sync.dma_start(
                                out=out_h.ap()[tb * 128:(tb + 1) * 128, :], in_=outt)

    nc.compile()
    return nc


def _prep_host(x, norm_w, norm_b, in_proj_w, conv_w, conv_b, gate_w, gate_b,
               out_proj_w):
    w1 = (in_proj_w * norm_w[None, :]).astype(np.float32)
    w1xT = np.ascontiguousarray(w1[:DI].T)           # [D, DI]
    w1zT = np.ascontiguousarray(w1[DI:].T)           # [D, DI]
    gwT = np.ascontiguousarray(gate_w.T)             # [DI, DI]
    opT = np.ascontiguousarray(out_proj_w.T)         # [DI, D]
    convw_r = np.ascontiguousarray(
        conv_w.reshape(KC, 128, 4).transpose(1, 0, 2).reshape(128, KC * 4))
    convb_r = np.ascontiguousarray(conv_b.reshape(KC, 128).T)
    gateb_r = np.ascontiguousarray(gate_b.reshape(KC, 128).T)
    normb_r = np.ascontiguousarray(norm_b.reshape(KD, 128).T)

    in_maps = []
    for core in range(8):
        b, j = core // 4, core % 4
        xs = np.zeros((NT, D), np.float32)
        start = j * CHUNK - W
        mask = np.ones((1, NT), np.float32)
        if j == 0:
            xs[W:] = x[b, 0:CHUNK]
            mask[0, :W] = 0.0
        else:
            xs[:] = x[b, start:start + NT]
        in_maps.append({
            "x": np.ascontiguousarray(xs), "w1x": w1xT, "w1z": w1zT,
            "gw": gwT, "opw": opT, "convw": convw_r, "convb": convb_r,
            "gateb": gateb_r, "normb": normb_r, "mask": mask,
        })
    return in_maps


def kernel(x, norm_w, norm_b, in_proj_w, conv_w, conv_b, gate_w, gate_b,
           out_proj_w, _trace=False, _collect=None):
    x = np.asarray(x, np.float32)
    if "nc" not in _cache:
        _cache["nc"] = _build()
    nc = _cache["nc"]
    in_maps = _prep_host(
        x, np.asarray(norm_w, np.float32), np.asarray(norm_b, np.float32),
        np.asarray(in_proj_w, np.float32), np.asarray(conv_w, np.float32),
        np.asarray(conv_b, np.float32), np.asarray(gate_w, np.float32),
        np.asarray(gate_b, np.float32), np.asarray(out_proj_w, np.float32))
    res = run_bass_kernel_spmd(nc, in_maps, core_ids=list(range(8)), trace=_trace)
    if _collect is not None:
        _collect.append(res)
    out = np.empty((B, L, D), np.float32)
    for core in range(8):
        b, j = core // 4, core % 4
        out[b, j * CHUNK:(j + 1) * CHUNK] = res.results[core]["out"]
    return out


# revision 19
# speedup vs baseline: 1.0309x; 1.0309x over previous
"""GatedLinearRecurrence Trainium2 kernel (8-core SPMD, Bass/Tile).

Strategy: shard (batch=2) x (4 sequence chunks of 1024 tokens) across 8 cores.
Each core processes 1152 tokens: a 128-token warm-up window (re-computed
redundantly; the recurrence decay e^{-~100} makes carry-in truncation error
~1e-24) followed by its 1024 "main" tokens.  No collectives needed.

Per-core pipeline (channels-on-partitions, tokens-on-free layout):
  LN(x) [t,d] -> PE-transpose -> x̂T [d,t] -> in_proj (f32r matmul)
  -> causal depthwise conv (4 shifted tensor_scalar ops) -> silu -> mask
  -> gate matmul -> sigmoid -> b=-(1-a)*xc -> tensor_tensor_scan (h=-h)
  -> y*silu(z) -> out_proj -> residual subtract -> out [t,dm].

The sign trick: scan data1 = (a-1)*x_conv = -b gives -h; -h*silu(z) = -yg;
out = x - matmul(-yg) = x + proj(yg).

Scheduling notes: x̂T is stored in 384-column chunk tiles so in_proj can
start before layernorm finishes; PSUM evacuations ride the Scalar engine
(Identity/Copy activations) to unload DVE; the scan is chunked (chained via
`initial=`) and interleaved into the gate loop so out_proj follows with no
PE gap; scan outputs reuse the a-tile pool slots (same tag) to fit SBUF.
"""
import sys

for p in ("/opt/trn_rl_repo", "/root/.axon_site/_ro/trn_rl_repo"):
    if p not in sys.path:
        sys.path.insert(0, p)

import numpy as np

import concourse.bass as bass
import concourse.bacc as bacc
import concourse.tile as tile
import concourse.mybir as mybir
from concourse.bass_utils import run_bass_kernel_spmd
from concourse.masks import make_identity

F32 = mybir.dt.float32
F32R = mybir.dt.float32r
AF = mybir.ActivationFunctionType
OP = mybir.AluOpType

B, L, D = 2, 4096, 1024
DI = 2048            # d_inner
NT = 1152            # tokens per core (128 warm-up + 1024 main)
W = 128              # warm-up tokens
CHUNK = 1024
NTT = NT // 128      # 9 token tiles
KD = D // 128        # 8 k-tiles over d_model
KC = DI // 128       # 16 k-tiles over d_inner
TC = 384             # matmul N chunk (3 per core)
NTC = NT // TC
EPS = 1e-5

_cache = {}


def _build():
    nc = bacc.Bacc(None, target_bir_lowering=False)

    x_h = nc.dram_tensor("x", [NT, D], F32, kind="ExternalInput")
    w1x_h = nc.dram_tensor("w1x", [D, DI], F32, kind="ExternalInput")
    w1z_h = nc.dram_tensor("w1z", [D, DI], F32, kind="ExternalInput")
    gw_h = nc.dram_tensor("gw", [DI, DI], F32, kind="ExternalInput")
    op_h = nc.dram_tensor("opw", [DI, D], F32, kind="ExternalInput")
    convw_h = nc.dram_tensor("convw", [128, KC * 4], F32, kind="ExternalInput")
    convb_h = nc.dram_tensor("convb", [128, KC], F32, kind="ExternalInput")
    gateb_h = nc.dram_tensor("gateb", [128, KC], F32, kind="ExternalInput")
    normb_h = nc.dram_tensor("normb", [128, KD], F32, kind="ExternalInput")
    mask_h = nc.dram_tensor("mask", [1, NT], F32, kind="ExternalInput")
    out_h = nc.dram_tensor("out", [CHUNK, D], F32, kind="ExternalOutput")
    z_h = nc.dram_tensor("z_scratch", [KC, 128, NT], F32, kind="Internal")
    yg_h = nc.dram_tensor("yg_scratch", [KC, 128, CHUNK], F32, kind="Internal")

    with tile.TileContext(nc) as tc:
        with tc.tile_pool(name="consts", bufs=1) as consts:

            ident = consts.tile([128, 128], F32, name="ident")
            make_identity(nc, ident)
            mask_sb = consts.tile([128, W], F32R, name="mask_sb")
            nc.gpsimd.dma_start(
                out=mask_sb,
                in_=bass.AP(tensor=mask_h, offset=0, ap=[[0, 128], [1, W]]).bitcast(F32R),
            )
            convw = consts.tile([128, KC * 4], F32, name="convw")
            nc.gpsimd.dma_start(out=convw, in_=convw_h.ap())
            convb = consts.tile([128, KC], F32, name="convb")
            nc.gpsimd.dma_start(out=convb, in_=convb_h.ap())
            gateb = consts.tile([128, KC], F32, name="gateb")
            nc.gpsimd.dma_start(out=gateb, in_=gateb_h.ap())
            normb = consts.tile([128, KD], F32, name="normb")
            nc.gpsimd.dma_start(out=normb, in_=normb_h.ap())
            eps_t = consts.tile([128, 1], F32, name="eps_t")
            nc.vector.memset(eps_t, EPS)

            with tc.tile_pool(name="xcp", bufs=1) as xcp:
                xc = [xcp.tile([128, NT], F32R, name=f"xct{e}") for e in range(KC)]

                # ---- S1-S3: LN, transpose, in_proj (x & z), conv, silu ----
                with tc.tile_pool(name="xT", bufs=1) as xTp, \
                     tc.tile_pool(name="s1roll", bufs=2) as s1r, \
                     tc.tile_pool(name="stat", bufs=3) as stp, \
                     tc.tile_pool(name="w1s", bufs=3) as ws, \
                     tc.tile_pool(name="zr", bufs=3) as zrp, \
                     tc.tile_pool(name="psmm", bufs=4, space="PSUM") as psmm, \
                     tc.tile_pool(name="pstr", bufs=2, space="PSUM") as pstr:

                    # x-hat-T chunk tiles [d-tile][t-chunk]: finer deps, so
                    # the first in_proj matmuls start after 3 LN iterations.
                    xT = [[xTp.tile([128, TC], F32R, name=f"xTt{d_}_{c_}")
                           for c_ in range(NTC)] for d_ in range(KD)]

                    for it in range(NTT):
                        tc3, col = it // 3, (it % 3) * 128
                        xt = s1r.tile([128, D], F32, tag="xt", name="xt")
                        nc.sync.dma_start(out=xt, in_=x_h.ap()[it * 128:(it + 1) * 128, :])
                        stats = stp.tile([128, 2, 6], F32, tag="stats", name="stats")
                        nc.vector.bn_stats(out=stats[:, 0, :], in_=xt[:, 0:512])
                        nc.vector.bn_stats(out=stats[:, 1, :], in_=xt[:, 512:1024])
                        mv = stp.tile([128, 2], F32, tag="mv", name="mv")
                        nc.vector.bn_aggr(out=mv, in_=stats)
                        rstd = stp.tile([128, 1], F32, tag="rstd", name="rstd")
                        nc.scalar.activation(out=rstd, in_=mv[:, 1:2], func=AF.Sqrt,
                                             bias=eps_t, scale=1.0)
                        nc.vector.reciprocal(out=rstd, in_=rstd)
                        nc.vector.tensor_scalar(out=xt, in0=xt, scalar1=mv[:, 0:1],
                                                scalar2=rstd, op0=OP.subtract, op1=OP.mult)
                        for d_ in range(KD):
                            pst = pstr.tile([128, 128], F32, tag="tr", name="pst")
                            nc.tensor.transpose(pst, xt[:, d_ * 128:(d_ + 1) * 128], ident)
                            # evac + norm_b on the Scalar engine
                            nc.scalar.activation(
                                out=xT[d_][tc3][:, col:col + 128], in_=pst,
                                func=AF.Identity, bias=normb[:, d_:d_ + 1], scale=1.0)

                    # in_proj x-half + conv + silu + warm-up mask.  The first
                    # 4 ets interleave their t-chunks so the PE never waits on
                    # layernorm chunks still in flight.
                    NW = 4
                    order = [(e, c) for c in range(NTC) for e in range(NW)]
                    order += [(e, c) for e in range(NW, KC) for c in range(NTC)]
                    wts, xins = {}, {}

                    def s2_chain(et, tc3):
                        if tc3 == 0:
                            wt = ws.tile([128, KD, 128], F32R, tag="w1",
                                         bufs=6, name=f"wt{et}")
                            nc.sync.dma_start(
                                out=wt,
                                in_=w1x_h.ap()[:, et * 128:(et + 1) * 128]
                                .rearrange("(kt p) e -> p kt e", p=128).bitcast(F32R))
                            wts[et] = wt
                            xin = s1r.tile([128, NT + 3], F32, tag="xin",
                                           bufs=NW + 2, name=f"xin{et}")
                            nc.vector.memset(xin[:, 0:3], 0.0)
                            xins[et] = xin
                        ps = psmm.tile([128, TC], F32, tag="mm", name="ps")
                        for kt in range(KD):
                            nc.tensor.matmul(
                                ps, wts[et][:, kt, :], xT[kt][tc3],
                                start=(kt == 0), stop=(kt == KD - 1))
                        nc.scalar.copy(
                            out=xins[et][:, 3 + tc3 * TC: 3 + (tc3 + 1) * TC], in_=ps)
                        if tc3 == NTC - 1:
                            xin = xins.pop(et)
                            tmp = s1r.tile([128, NT], F32, tag="ctmp", name="ctmp")
                            nc.vector.tensor_scalar_mul(
                                tmp, xin[:, 0:NT], convw[:, et * 4:et * 4 + 1])
                            for k in range(1, 4):
                                nc.vector.scalar_tensor_tensor(
                                    out=tmp, in0=xin[:, k:k + NT],
                                    scalar=convw[:, et * 4 + k:et * 4 + k + 1],
                                    in1=tmp, op0=OP.mult, op1=OP.add)
                            nc.scalar.activation(out=xc[et], in_=tmp, func=AF.Silu,
                                                 bias=convb[:, et:et + 1], scale=1.0)
                            # mask is non-unit only on the warm-up columns
                            nc.vector.tensor_mul(
                                xc[et][:, 0:W], xc[et][:, 0:W], mask_sb)

                    for et, tc3 in order:
                        s2_chain(et, tc3)

                    # in_proj z-half + silu -> HBM scratch
                    for et in range(KC):
                        wt = ws.tile([128, KD, 128], F32R, tag="w1", bufs=6, name="wtz")
                        nc.sync.dma_start(
                            out=wt,
                            in_=w1z_h.ap()[:, et * 128:(et + 1) * 128]
                            .rearrange("(kt p) e -> p kt e", p=128).bitcast(F32R))
                        for tc3 in range(NTC):
                            ps = psmm.tile([128, TC], F32, tag="mm", name="psz")
                            for kt in range(KD):
                                nc.tensor.matmul(
                                    ps, wt[:, kt, :], xT[kt][tc3],
                                    start=(kt == 0), stop=(kt == KD - 1))
                            zroll = zrp.tile([128, TC], F32, tag="zr", name="zroll")
                            nc.scalar.activation(out=zroll, in_=ps, func=AF.Silu)
                            nc.sync.dma_start(
                                out=z_h.ap()[et, :, tc3 * TC:(tc3 + 1) * TC], in_=zroll)

                # ---- S4-S6: gate matmul, sigmoid, chunked scan, y*silu(z) ----
                with tc.tile_pool(name="gws", bufs=3) as gs, \
                     tc.tile_pool(name="ay", bufs=4) as ayp, \
                     tc.tile_pool(name="s6roll", bufs=3) as s6r, \
                     tc.tile_pool(name="psg", bufs=4, space="PSUM") as psg:

                    for et in range(KC):
                        gt = gs.tile([128, KC, 128], F32R, tag="gw", name="gt")
                        nc.gpsimd.dma_start(
                            out=gt,
                            in_=gw_h.ap()[:, et * 128:(et + 1) * 128]
                            .rearrange("(kt p) e -> p kt e", p=128).bitcast(F32R))
                        zl = s6r.tile([128, CHUNK], F32, tag="zl", name="zl")
                        nc.gpsimd.dma_start(out=zl, in_=z_h.ap()[et, :, W:NT])
                        a_t = ayp.tile([128, NT], F32R, tag="ay", name=f"at{et}")
                        for tc3 in range(NTC):
                            ps = psg.tile([128, TC], F32, tag="mm", name="psg")
                            for kt in range(KC):
                                nc.tensor.matmul(
                                    ps, gt[:, kt, :], xc[kt][:, tc3 * TC:(tc3 + 1) * TC],
                                    start=(kt == 0), stop=(kt == KC - 1))
                            nc.scalar.activation(
                                out=a_t[:, tc3 * TC:(tc3 + 1) * TC], in_=ps,
                                func=AF.Sigmoid, bias=gateb[:, et:et + 1], scale=1.0)
                        # chunked scan chained via initial=, interleaved with
                        # the gate matmuls of later et (xc stays intact);
                        # -yg spills to HBM and is re-gathered by out_proj.
                        y_t = s6r.tile([128, NT], F32, tag="y", name=f"yt{et}")
                        for tc3 in range(NTC):
                            bt = s6r.tile([128, TC], F32, tag="bt", name="bt")
                            nc.vector.scalar_tensor_tensor(
                                out=bt, in0=a_t[:, tc3 * TC:(tc3 + 1) * TC], scalar=1.0,
                                in1=xc[et][:, tc3 * TC:(tc3 + 1) * TC],
                                op0=OP.subtract, op1=OP.mult)
                            init = 0.0 if tc3 == 0 else y_t[:, tc3 * TC - 1:tc3 * TC]
                            nc.vector.tensor_tensor_scan(
                                out=y_t[:, tc3 * TC:(tc3 + 1) * TC],
                                data0=a_t[:, tc3 * TC:(tc3 + 1) * TC],
                                data1=bt, initial=init, op0=OP.mult, op1=OP.add)
                        nc.vector.tensor_mul(y_t[:, W:NT], y_t[:, W:NT], zl)  # -yg
                        nc.sync.dma_start(out=yg_h.ap()[et], in_=y_t[:, W:NT])

            # ---- S7: out_proj + residual (xc pool closed; all 128 yg
            # gather tiles held resident; kt-major accumulation over 8
            # PSUM banks so gathers chase the spills with no barrier) ----
            NTB = CHUNK // 128
            with tc.tile_pool(name="opw", bufs=1) as opp, \
                 tc.tile_pool(name="ygp", bufs=1) as ygp, \
                 tc.tile_pool(name="s7roll", bufs=4) as s7r, \
                 tc.tile_pool(name="s7res", bufs=1) as s7x, \
                 tc.tile_pool(name="psop", bufs=8, space="PSUM") as psop:
                opt = [opp.tile([128, D], F32R, name=f"opt{k}") for k in range(KC)]
                for kt in range(KC):
                    nc.gpsimd.dma_start(
                        out=opt[kt], in_=op_h.ap()[kt * 128:(kt + 1) * 128, :].bitcast(F32R))
                ygt = [[None] * NTB for _ in range(KC)]
                for kt in range(KC):
                    for tb in range(NTB):
                        g = ygp.tile([128, 128], F32R, name=f"ygt{kt}_{tb}")
                        nc.gpsimd.dma_start(
                            out=g,
                            in_=yg_h.ap()[kt, :, tb * 128:(tb + 1) * 128].bitcast(F32R))
                        ygt[kt][tb] = g
                xres = [s7x.tile([128, D], F32, name=f"xres{tb}") for tb in range(NTB)]
                for tb in range(NTB):
                    nc.sync.dma_start(
                        out=xres[tb], in_=x_h.ap()[W + tb * 128:W + (tb + 1) * 128, :])
                for nb in range(2):
                    pss = [psop.tile([128, 512], F32, tag="op", name=f"pso{tb}")
                           for tb in range(NTB)]
                    for kt in range(KC):
                        for tb in range(NTB):
                            nc.tensor.matmul(
                                pss[tb], ygt[kt][tb],
                                opt[kt][:, nb * 512:(nb + 1) * 512],
                                start=(kt == 0), stop=(kt == KC - 1))
                    for tb in range(NTB):
                        oh = s7r.tile([128, 512], F32, tag="oh", name="oh")
                        nc.vector.tensor_sub(
                            oh, xres[tb][:, nb * 512:(nb + 1) * 512], pss[tb])
                        nc.sync.dma_start(
                            out=out_h.ap()[tb * 128:(tb + 1) * 128,
                                           nb * 512:(nb + 1) * 512], in_=oh)

    nc.compile()
    return nc


def _prep_host(x, norm_w, norm_b, in_proj_w, conv_w, conv_b, gate_w, gate_b,
               out_proj_w):
    w1 = (in_proj_w * norm_w[None, :]).astype(np.float32)
    w1xT = np.ascontiguousarray(w1[:DI].T)           # [D, DI]
    w1zT = np.ascontiguousarray(w1[DI:].T)           # [D, DI]
    gwT = np.ascontiguousarray(gate_w.T)             # [DI, DI]
    opT = np.ascontiguousarray(out_proj_w.T)         # [DI, D]
    convw_r = np.ascontiguousarray(
        conv_w.reshape(KC, 128, 4).transpose(1, 0, 2).reshape(128, KC * 4))
    convb_r = np.ascontiguousarray(conv_b.reshape(KC, 128).T)
    gateb_r = np.ascontiguousarray(gate_b.reshape(KC, 128).T)
    normb_r = np.ascontiguousarray(norm_b.reshape(KD, 128).T)

    in_maps = []
    for core in range(8):
        b, j = core // 4, core % 4
        xs = np.zeros((NT, D), np.float32)
        start = j * CHUNK - W
        mask = np.ones((1, NT), np.float32)
        if j == 0:
            xs[W:] = x[b, 0:CHUNK]
            mask[0, :W] = 0.0
        else:
            xs[:] = x[b, start:start + NT]
        in_maps.append({
            "x": np.ascontiguousarray(xs), "w1x": w1xT, "w1z": w1zT,
            "gw": gwT, "opw": opT, "convw": convw_r, "convb": convb_r,
            "gateb": gateb_r, "normb": normb_r, "mask": mask,
        })
    return in_maps


def kernel(x, norm_w, norm_b, in_proj_w, conv_w, conv_b, gate_w, gate_b,
           out_proj_w, _trace=False, _collect=None):
    x = np.asarray(x, np.float32)
    if "nc" not in _cache:
        _cache["nc"] = _build()
    nc = _cache["nc"]
    in_maps = _prep_host(
        x, np.asarray(norm_w, np.float32), np.asarray(norm_b, np.float32),
        np.asarray(in_proj_w, np.float32), np.asarray(conv_w, np.float32),
        np.asarray(conv_b, np.float32), np.asarray(gate_w, np.float32),
        np.asarray(gate_b, np.float32), np.asarray(out_proj_w, np.float32))
    res = run_bass_kernel_spmd(nc, in_maps, core_ids=list(range(8)), trace=_trace)
    if _collect is not None:
        _collect.append(res)
    out = np.empty((B, L, D), np.float32)
    for core in range(8):
        b, j = core // 4, core % 4
        out[b, j * CHUNK:(j + 1) * CHUNK] = res.results[core]["out"]
    return out


# revision 22
# speedup vs baseline: 1.1159x; 1.0824x over previous
"""GatedLinearRecurrence Trainium2 kernel (8-core SPMD, Bass/Tile).

Strategy: shard (batch=2) x (4 sequence chunks of 1024 tokens) across 8 cores.
Each core processes 1152 tokens: a 128-token warm-up window (re-computed
redundantly; the recurrence decay e^{-~100} makes carry-in truncation error
~1e-24) followed by its 1024 "main" tokens.  No collectives needed.

Per-core pipeline (channels-on-partitions, tokens-on-free layout):
  LN(x) [t,d] -> PE-transpose -> x̂T [d,t] -> in_proj (f32r matmul)
  -> causal depthwise conv (4 shifted tensor_scalar ops) -> silu -> mask
  -> gate matmul -> sigmoid -> b=-(1-a)*xc -> tensor_tensor_scan (h=-h)
  -> y*silu(z) -> out_proj -> residual subtract -> out [t,dm].

The sign trick: scan data1 = (a-1)*x_conv = -b gives -h; -h*silu(z) = -yg;
out = x - matmul(-yg) = x + proj(yg).

Scheduling notes: x̂T is stored in 384-column chunk tiles so in_proj can
start before layernorm finishes; PSUM evacuations ride the Scalar engine
(Identity/Copy activations) to unload DVE; the scan is chunked (chained via
`initial=`) and interleaved into the gate loop so out_proj follows with no
PE gap; scan outputs reuse the a-tile pool slots (same tag) to fit SBUF.
"""
import sys

for p in ("/opt/trn_rl_repo", "/root/.axon_site/_ro/trn_rl_repo"):
    if p not in sys.path:
        sys.path.insert(0, p)

import numpy as np

import concourse.bass as bass
import concourse.bacc as bacc
import concourse.tile as tile
import concourse.mybir as mybir
from concourse.bass_utils import run_bass_kernel_spmd
from concourse.masks import make_identity

F32 = mybir.dt.float32
F32R = mybir.dt.float32r
AF = mybir.ActivationFunctionType
OP = mybir.AluOpType

B, L, D = 2, 4096, 1024
DI = 2048            # d_inner
NT = 1152            # tokens per core (128 warm-up + 1024 main)
W = 128              # warm-up tokens
CHUNK = 1024
NTT = NT // 128      # 9 token tiles
KD = D // 128        # 8 k-tiles over d_model
KC = DI // 128       # 16 k-tiles over d_inner
TC = 384             # matmul N chunk (3 per core)
NTC = NT // TC
EPS = 1e-5

_cache = {}


def _build():
    nc = bacc.Bacc(None, target_bir_lowering=False)

    x_h = nc.dram_tensor("x", [NT, D], F32, kind="ExternalInput")
    w1x_h = nc.dram_tensor("w1x", [D, DI], F32, kind="ExternalInput")
    w1z_h = nc.dram_tensor("w1z", [D, DI], F32, kind="ExternalInput")
    gw_h = nc.dram_tensor("gw", [DI, DI], F32, kind="ExternalInput")
    op_h = nc.dram_tensor("opw", [DI, D], F32, kind="ExternalInput")
    convw_h = nc.dram_tensor("convw", [128, KC * 4], F32, kind="ExternalInput")
    convb_h = nc.dram_tensor("convb", [128, KC], F32, kind="ExternalInput")
    gateb_h = nc.dram_tensor("gateb", [128, KC], F32, kind="ExternalInput")
    normb_h = nc.dram_tensor("normb", [128, KD], F32, kind="ExternalInput")
    mask_h = nc.dram_tensor("mask", [1, NT], F32, kind="ExternalInput")
    out_h = nc.dram_tensor("out", [CHUNK, D], F32, kind="ExternalOutput")
    z_h = nc.dram_tensor("z_scratch", [KC, 128, NT], F32, kind="Internal")
    yg_h = nc.dram_tensor("yg_scratch", [KC, 128, CHUNK], F32, kind="Internal")

    with tile.TileContext(nc) as tc:
        with tc.tile_pool(name="consts", bufs=1) as consts:

            ident = consts.tile([128, 128], F32, name="ident")
            make_identity(nc, ident)
            mask_sb = consts.tile([128, W], F32R, name="mask_sb")
            nc.gpsimd.dma_start(
                out=mask_sb,
                in_=bass.AP(tensor=mask_h, offset=0, ap=[[0, 128], [1, W]]).bitcast(F32R),
            )
            convw = consts.tile([128, KC * 4], F32, name="convw")
            nc.gpsimd.dma_start(out=convw, in_=convw_h.ap())
            convb = consts.tile([128, KC], F32, name="convb")
            nc.gpsimd.dma_start(out=convb, in_=convb_h.ap())
            gateb = consts.tile([128, KC], F32, name="gateb")
            nc.gpsimd.dma_start(out=gateb, in_=gateb_h.ap())
            normb = consts.tile([128, KD], F32, name="normb")
            nc.gpsimd.dma_start(out=normb, in_=normb_h.ap())
            eps_t = consts.tile([128, 1], F32, name="eps_t")
            nc.vector.memset(eps_t, EPS)

            with tc.tile_pool(name="xcp", bufs=1) as xcp:
                xc = [xcp.tile([128, NT], F32R, name=f"xct{e}") for e in range(KC)]

                # ---- S1-S3: LN, transpose, in_proj (x & z), conv, silu ----
                with tc.tile_pool(name="xT", bufs=1) as xTp, \
                     tc.tile_pool(name="s1roll", bufs=2) as s1r, \
                     tc.tile_pool(name="stat", bufs=3) as stp, \
                     tc.tile_pool(name="w1s", bufs=3) as ws, \
                     tc.tile_pool(name="zr", bufs=3) as zrp, \
                     tc.tile_pool(name="psmm", bufs=4, space="PSUM") as psmm, \
                     tc.tile_pool(name="pstr", bufs=2, space="PSUM") as pstr:

                    # x-hat-T chunk tiles [d-tile][t-chunk]: finer deps, so
                    # the first in_proj matmuls start after 3 LN iterations.
                    xT = [[xTp.tile([128, TC], F32R, name=f"xTt{d_}_{c_}")
                           for c_ in range(NTC)] for d_ in range(KD)]

                    for it in range(NTT):
                        tc3, col = it // 3, (it % 3) * 128
                        xt = s1r.tile([128, D], F32, tag="xt", name="xt")
                        nc.sync.dma_start(out=xt, in_=x_h.ap()[it * 128:(it + 1) * 128, :])
                        stats = stp.tile([128, 2, 6], F32, tag="stats", name="stats")
                        nc.vector.bn_stats(out=stats[:, 0, :], in_=xt[:, 0:512])
                        nc.vector.bn_stats(out=stats[:, 1, :], in_=xt[:, 512:1024])
                        mv = stp.tile([128, 2], F32, tag="mv", name="mv")
                        nc.vector.bn_aggr(out=mv, in_=stats)
                        rstd = stp.tile([128, 1], F32, tag="rstd", name="rstd")
                        nc.scalar.activation(out=rstd, in_=mv[:, 1:2], func=AF.Sqrt,
                                             bias=eps_t, scale=1.0)
                        nc.vector.reciprocal(out=rstd, in_=rstd)
                        nc.vector.tensor_scalar(out=xt, in0=xt, scalar1=mv[:, 0:1],
                                                scalar2=rstd, op0=OP.subtract, op1=OP.mult)
                        for d_ in range(KD):
                            pst = pstr.tile([128, 128], F32, tag="tr", name="pst")
                            nc.tensor.transpose(pst, xt[:, d_ * 128:(d_ + 1) * 128], ident)
                            # evac + norm_b on the Scalar engine
                            nc.scalar.activation(
                                out=xT[d_][tc3][:, col:col + 128], in_=pst,
                                func=AF.Identity, bias=normb[:, d_:d_ + 1], scale=1.0)

                    # in_proj x-half + conv + silu + warm-up mask.  The first
                    # 4 ets interleave their t-chunks so the PE never waits on
                    # layernorm chunks still in flight.
                    NW = 4
                    order = [(e, c) for c in range(NTC) for e in range(NW)]
                    order += [(e, c) for e in range(NW, KC) for c in range(NTC)]
                    wts, xins = {}, {}

                    def s2_chain(et, tc3):
                        if tc3 == 0:
                            wt = ws.tile([128, KD, 128], F32R, tag="w1",
                                         bufs=6, name=f"wt{et}")
                            nc.sync.dma_start(
                                out=wt,
                                in_=w1x_h.ap()[:, et * 128:(et + 1) * 128]
                                .rearrange("(kt p) e -> p kt e", p=128).bitcast(F32R))
                            wts[et] = wt
                            xin = s1r.tile([128, NT + 3], F32, tag="xin",
                                           bufs=NW + 2, name=f"xin{et}")
                            nc.vector.memset(xin[:, 0:3], 0.0)
                            xins[et] = xin
                        ps = psmm.tile([128, TC], F32, tag="mm", name="ps")
                        for kt in range(KD):
                            nc.tensor.matmul(
                                ps, wts[et][:, kt, :], xT[kt][tc3],
                                start=(kt == 0), stop=(kt == KD - 1))
                        nc.scalar.copy(
                            out=xins[et][:, 3 + tc3 * TC: 3 + (tc3 + 1) * TC], in_=ps)
                        if tc3 == NTC - 1:
                            xin = xins.pop(et)
                            tmp = s1r.tile([128, NT], F32, tag="ctmp", name="ctmp")
                            nc.vector.tensor_scalar_mul(
                                tmp, xin[:, 0:NT], convw[:, et * 4:et * 4 + 1])
                            for k in range(1, 4):
                                nc.vector.scalar_tensor_tensor(
                                    out=tmp, in0=xin[:, k:k + NT],
                                    scalar=convw[:, et * 4 + k:et * 4 + k + 1],
                                    in1=tmp, op0=OP.mult, op1=OP.add)
                            nc.scalar.activation(out=xc[et], in_=tmp, func=AF.Silu,
                                                 bias=convb[:, et:et + 1], scale=1.0)
                            # mask is non-unit only on the warm-up columns
                            nc.vector.tensor_mul(
                                xc[et][:, 0:W], xc[et][:, 0:W], mask_sb)

                    for et, tc3 in order:
                        s2_chain(et, tc3)

                    # in_proj z-half + silu -> HBM scratch
                    for et in range(KC):
                        wt = ws.tile([128, KD, 128], F32R, tag="w1", bufs=6, name="wtz")
                        nc.sync.dma_start(
                            out=wt,
                            in_=w1z_h.ap()[:, et * 128:(et + 1) * 128]
                            .rearrange("(kt p) e -> p kt e", p=128).bitcast(F32R))
                        for tc3 in range(NTC):
                            ps = psmm.tile([128, TC], F32, tag="mm", name="psz")
                            for kt in range(KD):
                                nc.tensor.matmul(
                                    ps, wt[:, kt, :], xT[kt][tc3],
                                    start=(kt == 0), stop=(kt == KD - 1))
                            zroll = zrp.tile([128, TC], F32, tag="zr", name="zroll")
                            nc.scalar.activation(out=zroll, in_=ps, func=AF.Silu)
                            nc.sync.dma_start(
                                out=z_h.ap()[et, :, tc3 * TC:(tc3 + 1) * TC], in_=zroll)

                # ---- S4-S6: gate matmul, sigmoid, chunked scan, y*silu(z).
                # y is kept in SBUF as chunk tiles whose column slices feed
                # out_proj directly as stationary operands - no HBM roundtrip.
                with tc.tile_pool(name="yp", bufs=1) as yp:
                    ych = [[None] * NTC for _ in range(KC)]
                    with tc.tile_pool(name="gws", bufs=2) as gs, \
                         tc.tile_pool(name="ach", bufs=4) as ayp, \
                         tc.tile_pool(name="s6roll", bufs=4) as s6r, \
                         tc.tile_pool(name="psg", bufs=4, space="PSUM") as psg:

                        for et in range(KC):
                            gt = gs.tile([128, KC, 128], F32R, tag="gw", name="gt")
                            nc.gpsimd.dma_start(
                                out=gt,
                                in_=gw_h.ap()[:, et * 128:(et + 1) * 128]
                                .rearrange("(kt p) e -> p kt e", p=128).bitcast(F32R))
                            zls = []
                            for tc3 in range(NTC):
                                lo = max(tc3 * TC, W)
                                zl = s6r.tile([128, (tc3 + 1) * TC - lo], F32,
                                              tag="zl", name="zl")
                                nc.gpsimd.dma_start(
                                    out=zl, in_=z_h.ap()[et, :, lo:(tc3 + 1) * TC])
                                zls.append(zl)
                            prev_y = None
                            for tc3 in range(NTC):
                                a_t = ayp.tile([128, TC], F32R, tag="ach", name="ach")
                                ps = psg.tile([128, TC], F32, tag="mm", name="psgt")
                                for kt in range(KC):
                                    nc.tensor.matmul(
                                        ps, gt[:, kt, :],
                                        xc[kt][:, tc3 * TC:(tc3 + 1) * TC],
                                        start=(kt == 0), stop=(kt == KC - 1))
                                nc.scalar.activation(
                                    out=a_t, in_=ps,
                                    func=AF.Sigmoid, bias=gateb[:, et:et + 1], scale=1.0)
                                bt = s6r.tile([128, TC], F32, tag="bt", name="bt")
                                nc.vector.scalar_tensor_tensor(
                                    out=bt, in0=a_t, scalar=1.0,
                                    in1=xc[et][:, tc3 * TC:(tc3 + 1) * TC],
                                    op0=OP.subtract, op1=OP.mult)
                                y_t = yp.tile([128, TC], F32R, name=f"y{et}_{tc3}")
                                init = 0.0 if tc3 == 0 else prev_y[:, TC - 1:TC]
                                nc.vector.tensor_tensor_scan(
                                    out=y_t, data0=a_t, data1=bt, initial=init,
                                    op0=OP.mult, op1=OP.add)
                                ych[et][tc3] = y_t
                                prev_y = y_t
                            # -yg: multiply after the carry chain is complete
                            for tc3 in range(NTC):
                                lo = max(tc3 * TC, W) - tc3 * TC
                                nc.vector.tensor_mul(
                                    ych[et][tc3][:, lo:TC],
                                    ych[et][tc3][:, lo:TC], zls[tc3])

                    # ---- S7: out_proj + residual.  y chunk slices are the
                    # stationary operands; kt-major accumulation, two tb-half
                    # passes of 8 PSUM banks; opt streamed per (pass, kt). ----
                    NTB = CHUNK // 128

                    def yslice(kt, tb):
                        col = W + tb * 128          # absolute column in [0, NT)
                        tc3, off = col // TC, col % TC
                        return ych[kt][tc3][:, off:off + 128]

                    with tc.tile_pool(name="ops", bufs=3) as opp, \
                         tc.tile_pool(name="s7roll", bufs=4) as s7r, \
                         tc.tile_pool(name="s7res", bufs=5) as s7x, \
                         tc.tile_pool(name="psop", bufs=8, space="PSUM") as psop:
                        for half in range(2):
                            tbs = range(half * 4, half * 4 + 4)
                            xres = {}
                            for tb in tbs:
                                xres[tb] = s7x.tile([128, D], F32, tag="xres",
                                                    name=f"xres{tb}")
                                nc.sync.dma_start(
                                    out=xres[tb],
                                    in_=x_h.ap()[W + tb * 128:W + (tb + 1) * 128, :])
                            pss = {(tb, nb): psop.tile([128, 512], F32, tag="op",
                                                       name=f"pso{tb}_{nb}")
                                   for tb in tbs for nb in range(2)}
                            for kt in range(KC):
                                opt = opp.tile([128, D], F32R, tag="opw", name="opt")
                                nc.sync.dma_start(
                                    out=opt,
                                    in_=op_h.ap()[kt * 128:(kt + 1) * 128, :]
                                    .bitcast(F32R))
                                for tb in tbs:
                                    ys = yslice(kt, tb)
                                    for nb in range(2):
                                        nc.tensor.matmul(
                                            pss[(tb, nb)], ys,
                                            opt[:, nb * 512:(nb + 1) * 512],
                                            start=(kt == 0), stop=(kt == KC - 1))
                            for tb in tbs:
                                for nb in range(2):
                                    oh = s7r.tile([128, 512], F32, tag="oh", name="oh")
                                    nc.vector.tensor_sub(
                                        oh, xres[tb][:, nb * 512:(nb + 1) * 512],
                                        pss[(tb, nb)])
                                    nc.sync.dma_start(
                                        out=out_h.ap()[tb * 128:(tb + 1) * 128,
                                                       nb * 512:(nb + 1) * 512],
                                        in_=oh)

    nc.compile()
    return nc


def _prep_host(x, norm_w, norm_b, in_proj_w, conv_w, conv_b, gate_w, gate_b,
               out_proj_w):
    w1 = (in_proj_w * norm_w[None, :]).astype(np.float32)
    w1xT = np.ascontiguousarray(w1[:DI].T)           # [D, DI]
    w1zT = np.ascontiguousarray(w1[DI:].T)           # [D, DI]
    gwT = np.ascontiguousarray(gate_w.T)             # [DI, DI]
    opT = np.ascontiguousarray(out_proj_w.T)         # [DI, D]
    convw_r = np.ascontiguousarray(
        conv_w.reshape(KC, 128, 4).transpose(1, 0, 2).reshape(128, KC * 4))
    convb_r = np.ascontiguousarray(conv_b.reshape(KC, 128).T)
    gateb_r = np.ascontiguousarray(gate_b.reshape(KC, 128).T)
    normb_r = np.ascontiguousarray(norm_b.reshape(KD, 128).T)

    in_maps = []
    for core in range(8):
        b, j = core // 4, core % 4
        xs = np.zeros((NT, D), np.float32)
        start = j * CHUNK - W
        mask = np.ones((1, NT), np.float32)
        if j == 0:
            xs[W:] = x[b, 0:CHUNK]
            mask[0, :W] = 0.0
        else:
            xs[:] = x[b, start:start + NT]
        in_maps.append({
            "x": np.ascontiguousarray(xs), "w1x": w1xT, "w1z": w1zT,
            "gw": gwT, "opw": opT, "convw": convw_r, "convb": convb_r,
            "gateb": gateb_r, "normb": normb_r, "mask": mask,
        })
    return in_maps


def kernel(x, norm_w, norm_b, in_proj_w, conv_w, conv_b, gate_w, gate_b,
           out_proj_w, _trace=False, _collect=None):
    x = np.asarray(x, np.float32)
    if "nc" not in _cache:
        _cache["nc"] = _build()
    nc = _cache["nc"]
    in_maps = _prep_host(
        x, np.asarray(norm_w, np.float32), np.asarray(norm_b, np.float32),
        np.asarray(in_proj_w, np.float32), np.asarray(conv_w, np.float32),
        np.asarray(conv_b, np.float32), np.asarray(gate_w, np.float32),
        np.asarray(gate_b, np.float32), np.asarray(out_proj_w, np.float32))
    res = run_bass_kernel_spmd(nc, in_maps, core_ids=list(range(8)), trace=_trace)
    if _collect is not None:
        _collect.append(res)
    out = np.empty((B, L, D), np.float32)
    for core in range(8):
        b, j = core // 4, core % 4
        out[b, j * CHUNK:(j + 1) * CHUNK] = res.results[core]["out"]
    return out


# revision 23
# speedup vs baseline: 1.1849x; 1.0619x over previous
"""GatedLinearRecurrence Trainium2 kernel (8-core SPMD, Bass/Tile).

Strategy: shard (batch=2) x (4 sequence chunks of 1024 tokens) across 8 cores.
Each core processes 1152 tokens: a 128-token warm-up window (re-computed
redundantly; the recurrence decay e^{-~100} makes carry-in truncation error
~1e-24) followed by its 1024 "main" tokens.  No collectives needed.

Per-core pipeline (channels-on-partitions, tokens-on-free layout):
  LN(x) [t,d] -> PE-transpose -> x̂T [d,t] -> in_proj (f32r matmul)
  -> causal depthwise conv (4 shifted tensor_scalar ops) -> silu -> mask
  -> gate matmul -> sigmoid -> b=-(1-a)*xc -> tensor_tensor_scan (h=-h)
  -> y*silu(z) -> out_proj -> residual subtract -> out [t,dm].

The sign trick: scan data1 = (a-1)*x_conv = -b gives -h; -h*silu(z) = -yg;
out = x - matmul(-yg) = x + proj(yg).

Scheduling notes: x̂T is stored in 384-column chunk tiles so in_proj can
start before layernorm finishes; PSUM evacuations ride the Scalar engine
(Identity/Copy activations) to unload DVE; the scan is chunked (chained via
`initial=`) and interleaved into the gate loop so out_proj follows with no
PE gap; scan outputs reuse the a-tile pool slots (same tag) to fit SBUF.
"""
import sys

for p in ("/opt/trn_rl_repo", "/root/.axon_site/_ro/trn_rl_repo"):
    if p not in sys.path:
        sys.path.insert(0, p)

import numpy as np

import concourse.bass as bass
import concourse.bacc as bacc
import concourse.tile as tile
import concourse.mybir as mybir
from concourse.bass_utils import run_bass_kernel_spmd
from concourse.masks import make_identity

F32 = mybir.dt.float32
F32R = mybir.dt.float32r
AF = mybir.ActivationFunctionType
OP = mybir.AluOpType

B, L, D = 2, 4096, 1024
DI = 2048            # d_inner
NT = 1152            # tokens per core (128 warm-up + 1024 main)
W = 128              # warm-up tokens
CHUNK = 1024
NTT = NT // 128      # 9 token tiles
KD = D // 128        # 8 k-tiles over d_model
KC = DI // 128       # 16 k-tiles over d_inner
TC = 384             # matmul N chunk (3 per core)
NTC = NT // TC
EPS = 1e-5

_cache = {}


def _build():
    nc = bacc.Bacc(None, target_bir_lowering=False)

    x_h = nc.dram_tensor("x", [NT, D], F32, kind="ExternalInput")
    w1x_h = nc.dram_tensor("w1x", [D, DI], F32, kind="ExternalInput")
    w1z_h = nc.dram_tensor("w1z", [D, DI], F32, kind="ExternalInput")
    gw_h = nc.dram_tensor("gw", [DI, DI], F32, kind="ExternalInput")
    op_h = nc.dram_tensor("opw", [DI, D], F32, kind="ExternalInput")
    convw_h = nc.dram_tensor("convw", [128, KC * 4], F32, kind="ExternalInput")
    convb_h = nc.dram_tensor("convb", [128, KC], F32, kind="ExternalInput")
    gateb_h = nc.dram_tensor("gateb", [128, KC], F32, kind="ExternalInput")
    normb_h = nc.dram_tensor("normb", [128, KD], F32, kind="ExternalInput")
    mask_h = nc.dram_tensor("mask", [1, NT], F32, kind="ExternalInput")
    out_h = nc.dram_tensor("out", [CHUNK, D], F32, kind="ExternalOutput")
    z_h = nc.dram_tensor("z_scratch", [KC, 128, NT], F32, kind="Internal")
    yg_h = nc.dram_tensor("yg_scratch", [KC, 128, CHUNK], F32, kind="Internal")

    with tile.TileContext(nc) as tc:
        with tc.tile_pool(name="consts", bufs=1) as consts:

            ident = consts.tile([128, 128], F32, name="ident")
            make_identity(nc, ident)
            mask_sb = consts.tile([128, W], F32R, name="mask_sb")
            nc.gpsimd.dma_start(
                out=mask_sb,
                in_=bass.AP(tensor=mask_h, offset=0, ap=[[0, 128], [1, W]]).bitcast(F32R),
            )
            convw = consts.tile([128, KC * 4], F32, name="convw")
            nc.gpsimd.dma_start(out=convw, in_=convw_h.ap())
            convb = consts.tile([128, KC], F32, name="convb")
            nc.gpsimd.dma_start(out=convb, in_=convb_h.ap())
            gateb = consts.tile([128, KC], F32, name="gateb")
            nc.gpsimd.dma_start(out=gateb, in_=gateb_h.ap())
            normb = consts.tile([128, KD], F32, name="normb")
            nc.gpsimd.dma_start(out=normb, in_=normb_h.ap())
            eps_t = consts.tile([128, 1], F32, name="eps_t")
            nc.vector.memset(eps_t, EPS)

            with tc.tile_pool(name="xcp", bufs=1) as xcp:
                xc = [xcp.tile([128, NT], F32R, name=f"xct{e}") for e in range(KC)]

                # ---- S1-S3: LN, transpose, in_proj (x & z), conv, silu ----
                with tc.tile_pool(name="xT", bufs=1) as xTp, \
                     tc.tile_pool(name="s1roll", bufs=2) as s1r, \
                     tc.tile_pool(name="stat", bufs=3) as stp, \
                     tc.tile_pool(name="w1s", bufs=3) as ws, \
                     tc.tile_pool(name="zr", bufs=4) as zrp, \
                     tc.tile_pool(name="psmm", bufs=4, space="PSUM") as psmm, \
                     tc.tile_pool(name="pstr", bufs=2, space="PSUM") as pstr:

                    # x-hat-T chunk tiles [d-tile][t-chunk]: finer deps, so
                    # the first in_proj matmuls start after 3 LN iterations.
                    xT = [[xTp.tile([128, TC], F32R, name=f"xTt{d_}_{c_}")
                           for c_ in range(NTC)] for d_ in range(KD)]

                    for it in range(NTT):
                        tc3, col = it // 3, (it % 3) * 128
                        xt = s1r.tile([128, D], F32, tag="xt", bufs=3, name="xt")
                        nc.sync.dma_start(out=xt, in_=x_h.ap()[it * 128:(it + 1) * 128, :])
                        stats = stp.tile([128, 2, 6], F32, tag="stats", name="stats")
                        nc.vector.bn_stats(out=stats[:, 0, :], in_=xt[:, 0:512])
                        nc.vector.bn_stats(out=stats[:, 1, :], in_=xt[:, 512:1024])
                        mv = stp.tile([128, 2], F32, tag="mv", name="mv")
                        nc.vector.bn_aggr(out=mv, in_=stats)
                        rstd = stp.tile([128, 1], F32, tag="rstd", name="rstd")
                        nc.scalar.activation(out=rstd, in_=mv[:, 1:2], func=AF.Sqrt,
                                             bias=eps_t, scale=1.0)
                        nc.vector.reciprocal(out=rstd, in_=rstd)
                        nc.vector.tensor_scalar(out=xt, in0=xt, scalar1=mv[:, 0:1],
                                                scalar2=rstd, op0=OP.subtract, op1=OP.mult)
                        for d_ in range(KD):
                            pst = pstr.tile([128, 128], F32, tag="tr", name="pst")
                            nc.tensor.transpose(pst, xt[:, d_ * 128:(d_ + 1) * 128], ident)
                            # evac + norm_b on the Scalar engine
                            nc.scalar.activation(
                                out=xT[d_][tc3][:, col:col + 128], in_=pst,
                                func=AF.Identity, bias=normb[:, d_:d_ + 1], scale=1.0)

                    # in_proj x-half + conv + silu + warm-up mask.  The first
                    # 4 ets interleave their t-chunks so the PE never waits on
                    # layernorm chunks still in flight.
                    NW = 4
                    order = [(e, c) for c in range(NTC) for e in range(NW)]
                    order += [(e, c) for e in range(NW, KC) for c in range(NTC)]
                    wts, xins = {}, {}

                    def s2_chain(et, tc3):
                        if tc3 == 0:
                            wt = ws.tile([128, KD, 128], F32R, tag="w1",
                                         bufs=6, name=f"wt{et}")
                            nc.sync.dma_start(
                                out=wt,
                                in_=w1x_h.ap()[:, et * 128:(et + 1) * 128]
                                .rearrange("(kt p) e -> p kt e", p=128).bitcast(F32R))
                            wts[et] = wt
                            xin = s1r.tile([128, NT + 3], F32, tag="xin",
                                           bufs=NW + 2, name=f"xin{et}")
                            nc.vector.memset(xin[:, 0:3], 0.0)
                            xins[et] = xin
                        ps = psmm.tile([128, TC], F32, tag="mm", name="ps")
                        for kt in range(KD):
                            nc.tensor.matmul(
                                ps, wts[et][:, kt, :], xT[kt][tc3],
                                start=(kt == 0), stop=(kt == KD - 1))
                        nc.scalar.copy(
                            out=xins[et][:, 3 + tc3 * TC: 3 + (tc3 + 1) * TC], in_=ps)
                        if tc3 == NTC - 1:
                            xin = xins.pop(et)
                            tmp = s1r.tile([128, NT], F32, tag="ctmp", name="ctmp")
                            nc.vector.tensor_scalar_mul(
                                tmp, xin[:, 0:NT], convw[:, et * 4:et * 4 + 1])
                            for k in range(1, 4):
                                nc.vector.scalar_tensor_tensor(
                                    out=tmp, in0=xin[:, k:k + NT],
                                    scalar=convw[:, et * 4 + k:et * 4 + k + 1],
                                    in1=tmp, op0=OP.mult, op1=OP.add)
                            nc.scalar.activation(out=xc[et], in_=tmp, func=AF.Silu,
                                                 bias=convb[:, et:et + 1], scale=1.0)
                            # mask is non-unit only on the warm-up columns
                            nc.vector.tensor_mul(
                                xc[et][:, 0:W], xc[et][:, 0:W], mask_sb)

                    for et, tc3 in order:
                        s2_chain(et, tc3)

                    # in_proj z-half + silu -> HBM scratch
                    for et in range(KC):
                        wt = ws.tile([128, KD, 128], F32R, tag="w1", bufs=6, name="wtz")
                        nc.sync.dma_start(
                            out=wt,
                            in_=w1z_h.ap()[:, et * 128:(et + 1) * 128]
                            .rearrange("(kt p) e -> p kt e", p=128).bitcast(F32R))
                        for tc3 in range(NTC):
                            ps = psmm.tile([128, TC], F32, tag="mm", name="psz")
                            for kt in range(KD):
                                nc.tensor.matmul(
                                    ps, wt[:, kt, :], xT[kt][tc3],
                                    start=(kt == 0), stop=(kt == KD - 1))
                            zroll = zrp.tile([128, TC], F32, tag="zr", name="zroll")
                            nc.scalar.activation(out=zroll, in_=ps, func=AF.Silu)
                            nc.sync.dma_start(
                                out=z_h.ap()[et, :, tc3 * TC:(tc3 + 1) * TC], in_=zroll)

                # ---- S4-S6: gate matmul, sigmoid, chunked scan, y*silu(z).
                # y is kept in SBUF as chunk tiles whose column slices feed
                # out_proj directly as stationary operands - no HBM roundtrip.
                with tc.tile_pool(name="yp", bufs=1) as yp:
                    ych = [[None] * NTC for _ in range(KC)]
                    with tc.tile_pool(name="gws", bufs=3) as gs, \
                         tc.tile_pool(name="ach", bufs=4) as ayp, \
                         tc.tile_pool(name="s6roll", bufs=4) as s6r, \
                         tc.tile_pool(name="psg", bufs=4, space="PSUM") as psg:

                        for et in range(KC):
                            gt = gs.tile([128, KC, 128], F32R, tag="gw", name="gt")
                            nc.gpsimd.dma_start(
                                out=gt,
                                in_=gw_h.ap()[:, et * 128:(et + 1) * 128]
                                .rearrange("(kt p) e -> p kt e", p=128).bitcast(F32R))
                            zls = []
                            for tc3 in range(NTC):
                                lo = max(tc3 * TC, W)
                                zl = s6r.tile([128, (tc3 + 1) * TC - lo], F32,
                                              tag="zl", name="zl")
                                nc.gpsimd.dma_start(
                                    out=zl, in_=z_h.ap()[et, :, lo:(tc3 + 1) * TC])
                                zls.append(zl)
                            prev_y = None
                            for tc3 in range(NTC):
                                a_t = ayp.tile([128, TC], F32R, tag="ach", name="ach")
                                ps = psg.tile([128, TC], F32, tag="mm", name="psgt")
                                for kt in range(KC):
                                    nc.tensor.matmul(
                                        ps, gt[:, kt, :],
                                        xc[kt][:, tc3 * TC:(tc3 + 1) * TC],
                                        start=(kt == 0), stop=(kt == KC - 1))
                                nc.scalar.activation(
                                    out=a_t, in_=ps,
                                    func=AF.Sigmoid, bias=gateb[:, et:et + 1], scale=1.0)
                                bt = s6r.tile([128, TC], F32, tag="bt", name="bt")
                                nc.vector.scalar_tensor_tensor(
                                    out=bt, in0=a_t, scalar=1.0,
                                    in1=xc[et][:, tc3 * TC:(tc3 + 1) * TC],
                                    op0=OP.subtract, op1=OP.mult)
                                y_t = yp.tile([128, TC], F32R, name=f"y{et}_{tc3}")
                                init = 0.0 if tc3 == 0 else prev_y[:, TC - 1:TC]
                                nc.vector.tensor_tensor_scan(
                                    out=y_t, data0=a_t, data1=bt, initial=init,
                                    op0=OP.mult, op1=OP.add)
                                ych[et][tc3] = y_t
                                prev_y = y_t
                            # -yg: multiply after the carry chain is complete
                            for tc3 in range(NTC):
                                lo = max(tc3 * TC, W) - tc3 * TC
                                nc.vector.tensor_mul(
                                    ych[et][tc3][:, lo:TC],
                                    ych[et][tc3][:, lo:TC], zls[tc3])

                    # ---- S7: out_proj + residual.  y chunk slices are the
                    # stationary operands; kt-major accumulation, two tb-half
                    # passes of 8 PSUM banks; opt streamed per (pass, kt). ----
                    NTB = CHUNK // 128

                    def yslice(kt, tb):
                        col = W + tb * 128          # absolute column in [0, NT)
                        tc3, off = col // TC, col % TC
                        return ych[kt][tc3][:, off:off + 128]

                    with tc.tile_pool(name="ops", bufs=3) as opp, \
                         tc.tile_pool(name="s7roll", bufs=4) as s7r, \
                         tc.tile_pool(name="s7res", bufs=5) as s7x, \
                         tc.tile_pool(name="psop", bufs=8, space="PSUM") as psop:
                        for half in range(2):
                            tbs = range(half * 4, half * 4 + 4)
                            xres = {}
                            for tb in tbs:
                                xres[tb] = s7x.tile([128, D], F32, tag="xres",
                                                    name=f"xres{tb}")
                                nc.sync.dma_start(
                                    out=xres[tb],
                                    in_=x_h.ap()[W + tb * 128:W + (tb + 1) * 128, :])
                            pss = {(tb, nb): psop.tile([128, 512], F32, tag="op",
                                                       name=f"pso{tb}_{nb}")
                                   for tb in tbs for nb in range(2)}
                            for kt in range(KC):
                                opt = opp.tile([128, D], F32R, tag="opw", name="opt")
                                nc.sync.dma_start(
                                    out=opt,
                                    in_=op_h.ap()[kt * 128:(kt + 1) * 128, :]
                                    .bitcast(F32R))
                                for tb in tbs:
                                    ys = yslice(kt, tb)
                                    for nb in range(2):
                                        nc.tensor.matmul(
                                            pss[(tb, nb)], ys,
                                            opt[:, nb * 512:(nb + 1) * 512],
                                            start=(kt == 0), stop=(kt == KC - 1))
                            for tb in tbs:
                                for nb in range(2):
                                    oh = s7r.tile([128, 512], F32, tag="oh", name="oh")
                                    nc.vector.tensor_sub(
                                        oh, xres[tb][:, nb * 512:(nb + 1) * 512],
                                        pss[(tb, nb)])
                                    nc.sync.dma_start(
                                        out=out_h.ap()[tb * 128:(tb + 1) * 128,
                                                       nb * 512:(nb + 1) * 512],
                                        in_=oh)

    nc.compile()
    return nc


def _prep_host(x, norm_w, norm_b, in_proj_w, conv_w, conv_b, gate_w, gate_b,
               out_proj_w):
    w1 = (in_proj_w * norm_w[None, :]).astype(np.float32)
    w1xT = np.ascontiguousarray(w1[:DI].T)           # [D, DI]
    w1zT = np.ascontiguousarray(w1[DI:].T)           # [D, DI]
    gwT = np.ascontiguousarray(gate_w.T)             # [DI, DI]
    opT = np.ascontiguousarray(out_proj_w.T)         # [DI, D]
    convw_r = np.ascontiguousarray(
        conv_w.reshape(KC, 128, 4).transpose(1, 0, 2).reshape(128, KC * 4))
    convb_r = np.ascontiguousarray(conv_b.reshape(KC, 128).T)
    gateb_r = np.ascontiguousarray(gate_b.reshape(KC, 128).T)
    normb_r = np.ascontiguousarray(norm_b.reshape(KD, 128).T)

    in_maps = []
    for core in range(8):
        b, j = core // 4, core % 4
        xs = np.zeros((NT, D), np.float32)
        start = j * CHUNK - W
        mask = np.ones((1, NT), np.float32)
        if j == 0:
            xs[W:] = x[b, 0:CHUNK]
            mask[0, :W] = 0.0
        else:
            xs[:] = x[b, start:start + NT]
        in_maps.append({
            "x": np.ascontiguousarray(xs), "w1x": w1xT, "w1z": w1zT,
            "gw": gwT, "opw": opT, "convw": convw_r, "convb": convb_r,
            "gateb": gateb_r, "normb": normb_r, "mask": mask,
        })
    return in_maps


def kernel(x, norm_w, norm_b, in_proj_w, conv_w, conv_b, gate_w, gate_b,
           out_proj_w, _trace=False, _collect=None):
    x = np.asarray(x, np.float32)
    if "nc" not in _cache:
        _cache["nc"] = _build()
    nc = _cache["nc"]
    in_maps = _prep_host(
        x, np.asarray(norm_w, np.float32), np.asarray(norm_b, np.float32),
        np.asarray(in_proj_w, np.float32), np.asarray(conv_w, np.float32),
        np.asarray(conv_b, np.float32), np.asarray(gate_w, np.float32),
        np.asarray(gate_b, np.float32), np.asarray(out_proj_w, np.float32))
    res = run_bass_kernel_spmd(nc, in_maps, core_ids=list(range(8)), trace=_trace)
    if _collect is not None:
        _collect.append(res)
    out = np.empty((B, L, D), np.float32)
    for core in range(8):
        b, j = core // 4, core % 4
        out[b, j * CHUNK:(j + 1) * CHUNK] = res.results[core]["out"]
    return out


# revision 24
# speedup vs baseline: 1.1971x; 1.0103x over previous
"""GatedLinearRecurrence Trainium2 kernel (8-core SPMD, Bass/Tile).

Strategy: shard (batch=2) x (4 sequence chunks of 1024 tokens) across 8 cores.
Each core processes 1152 tokens: a 128-token warm-up window (re-computed
redundantly; the recurrence decay e^{-~100} makes carry-in truncation error
~1e-24) followed by its 1024 "main" tokens.  No collectives needed.

Per-core pipeline (channels-on-partitions, tokens-on-free layout):
  LN(x) [t,d] -> PE-transpose -> x̂T [d,t] -> in_proj (f32r matmul)
  -> causal depthwise conv (4 shifted tensor_scalar ops) -> silu -> mask
  -> gate matmul -> sigmoid -> b=-(1-a)*xc -> tensor_tensor_scan (h=-h)
  -> y*silu(z) -> out_proj -> residual subtract -> out [t,dm].

The sign trick: scan data1 = (a-1)*x_conv = -b gives -h; -h*silu(z) = -yg;
out = x - matmul(-yg) = x + proj(yg).

Scheduling notes: x̂T is stored in 384-column chunk tiles so in_proj can
start before layernorm finishes; PSUM evacuations ride the Scalar engine
(Identity/Copy activations) to unload DVE; the scan is chunked (chained via
`initial=`) and interleaved into the gate loop so out_proj follows with no
PE gap; scan outputs reuse the a-tile pool slots (same tag) to fit SBUF.
"""
import sys

for p in ("/opt/trn_rl_repo", "/root/.axon_site/_ro/trn_rl_repo"):
    if p not in sys.path:
        sys.path.insert(0, p)

import numpy as np

import concourse.bass as bass
import concourse.bacc as bacc
import concourse.tile as tile
import concourse.mybir as mybir
from concourse.bass_utils import run_bass_kernel_spmd
from concourse.masks import make_identity

F32 = mybir.dt.float32
F32R = mybir.dt.float32r
AF = mybir.ActivationFunctionType
OP = mybir.AluOpType

B, L, D = 2, 4096, 1024
DI = 2048            # d_inner
NT = 1152            # tokens per core (128 warm-up + 1024 main)
W = 128              # warm-up tokens
CHUNK = 1024
NTT = NT // 128      # 9 token tiles
KD = D // 128        # 8 k-tiles over d_model
KC = DI // 128       # 16 k-tiles over d_inner
TC = 384             # matmul N chunk (3 per core)
NTC = NT // TC
EPS = 1e-5

_cache = {}


def _build():
    nc = bacc.Bacc(None, target_bir_lowering=False)

    x_h = nc.dram_tensor("x", [NT, D], F32, kind="ExternalInput")
    w1x_h = nc.dram_tensor("w1x", [D, DI], F32, kind="ExternalInput")
    w1z_h = nc.dram_tensor("w1z", [D, DI], F32, kind="ExternalInput")
    gw_h = nc.dram_tensor("gw", [DI, DI], F32, kind="ExternalInput")
    op_h = nc.dram_tensor("opw", [DI, D], F32, kind="ExternalInput")
    convw_h = nc.dram_tensor("convw", [128, KC * 4], F32, kind="ExternalInput")
    convb_h = nc.dram_tensor("convb", [128, KC], F32, kind="ExternalInput")
    gateb_h = nc.dram_tensor("gateb", [128, KC], F32, kind="ExternalInput")
    normb_h = nc.dram_tensor("normb", [128, KD], F32, kind="ExternalInput")
    mask_h = nc.dram_tensor("mask", [1, NT], F32, kind="ExternalInput")
    out_h = nc.dram_tensor("out", [CHUNK, D], F32, kind="ExternalOutput")
    z_h = nc.dram_tensor("z_scratch", [KC, 128, NT], F32, kind="Internal")
    yg_h = nc.dram_tensor("yg_scratch", [KC, 128, CHUNK], F32, kind="Internal")

    with tile.TileContext(nc) as tc:
        with tc.tile_pool(name="consts", bufs=1) as consts:

            ident = consts.tile([128, 128], F32, name="ident")
            make_identity(nc, ident)
            mask_sb = consts.tile([128, W], F32R, name="mask_sb")
            nc.gpsimd.dma_start(
                out=mask_sb,
                in_=bass.AP(tensor=mask_h, offset=0, ap=[[0, 128], [1, W]]).bitcast(F32R),
            )
            convw = consts.tile([128, KC * 4], F32, name="convw")
            nc.gpsimd.dma_start(out=convw, in_=convw_h.ap())
            convb = consts.tile([128, KC], F32, name="convb")
            nc.gpsimd.dma_start(out=convb, in_=convb_h.ap())
            gateb = consts.tile([128, KC], F32, name="gateb")
            nc.gpsimd.dma_start(out=gateb, in_=gateb_h.ap())
            normb = consts.tile([128, KD], F32, name="normb")
            nc.gpsimd.dma_start(out=normb, in_=normb_h.ap())
            eps_t = consts.tile([128, 1], F32, name="eps_t")
            nc.vector.memset(eps_t, EPS)

            with tc.tile_pool(name="xcp", bufs=1) as xcp:
                xc = [xcp.tile([128, NT], F32R, name=f"xct{e}") for e in range(KC)]

                # ---- S1-S3: LN, transpose, in_proj (x & z), conv, silu ----
                with tc.tile_pool(name="xT", bufs=1) as xTp, \
                     tc.tile_pool(name="s1roll", bufs=2) as s1r, \
                     tc.tile_pool(name="stat", bufs=3) as stp, \
                     tc.tile_pool(name="w1s", bufs=3) as ws, \
                     tc.tile_pool(name="zr", bufs=4) as zrp, \
                     tc.tile_pool(name="psmm", bufs=5, space="PSUM") as psmm, \
                     tc.tile_pool(name="pstr", bufs=2, space="PSUM") as pstr:

                    # x-hat-T chunk tiles [d-tile][t-chunk]: finer deps, so
                    # the first in_proj matmuls start after 3 LN iterations.
                    xT = [[xTp.tile([128, TC], F32R, name=f"xTt{d_}_{c_}")
                           for c_ in range(NTC)] for d_ in range(KD)]

                    for it in range(NTT):
                        tc3, col = it // 3, (it % 3) * 128
                        xt = s1r.tile([128, D], F32, tag="xt", bufs=3, name="xt")
                        nc.sync.dma_start(out=xt, in_=x_h.ap()[it * 128:(it + 1) * 128, :])
                        stats = stp.tile([128, 2, 6], F32, tag="stats", name="stats")
                        nc.vector.bn_stats(out=stats[:, 0, :], in_=xt[:, 0:512])
                        nc.vector.bn_stats(out=stats[:, 1, :], in_=xt[:, 512:1024])
                        mv = stp.tile([128, 2], F32, tag="mv", name="mv")
                        nc.vector.bn_aggr(out=mv, in_=stats)
                        rstd = stp.tile([128, 1], F32, tag="rstd", name="rstd")
                        nc.scalar.activation(out=rstd, in_=mv[:, 1:2], func=AF.Sqrt,
                                             bias=eps_t, scale=1.0)
                        nc.vector.reciprocal(out=rstd, in_=rstd)
                        nc.vector.tensor_scalar(out=xt, in0=xt, scalar1=mv[:, 0:1],
                                                scalar2=rstd, op0=OP.subtract, op1=OP.mult)
                        for d_ in range(KD):
                            pst = pstr.tile([128, 128], F32, tag="tr", name="pst")
                            nc.tensor.transpose(pst, xt[:, d_ * 128:(d_ + 1) * 128], ident)
                            # evac + norm_b on the Scalar engine
                            nc.scalar.activation(
                                out=xT[d_][tc3][:, col:col + 128], in_=pst,
                                func=AF.Identity, bias=normb[:, d_:d_ + 1], scale=1.0)

                    # in_proj x-half + conv + silu + warm-up mask.  The first
                    # 4 ets interleave their t-chunks so the PE never waits on
                    # layernorm chunks still in flight.
                    NW = 4
                    order = [(e, c) for c in range(NTC) for e in range(NW)]
                    order += [(e, c) for e in range(NW, KC) for c in range(NTC)]
                    wts, xins = {}, {}

                    def s2_chain(et, tc3):
                        if tc3 == 0:
                            wt = ws.tile([128, KD, 128], F32R, tag="w1",
                                         bufs=6, name=f"wt{et}")
                            nc.sync.dma_start(
                                out=wt,
                                in_=w1x_h.ap()[:, et * 128:(et + 1) * 128]
                                .rearrange("(kt p) e -> p kt e", p=128).bitcast(F32R))
                            wts[et] = wt
                            xin = s1r.tile([128, NT + 3], F32, tag="xin",
                                           bufs=NW + 2, name=f"xin{et}")
                            nc.vector.memset(xin[:, 0:3], 0.0)
                            xins[et] = xin
                        ps = psmm.tile([128, TC], F32, tag="mm", name="ps")
                        for kt in range(KD):
                            nc.tensor.matmul(
                                ps, wts[et][:, kt, :], xT[kt][tc3],
                                start=(kt == 0), stop=(kt == KD - 1))
                        nc.scalar.copy(
                            out=xins[et][:, 3 + tc3 * TC: 3 + (tc3 + 1) * TC], in_=ps)
                        if tc3 == NTC - 1:
                            xin = xins.pop(et)
                            tmp = s1r.tile([128, NT], F32, tag="ctmp", name="ctmp")
                            nc.vector.tensor_scalar_mul(
                                tmp, xin[:, 0:NT], convw[:, et * 4:et * 4 + 1])
                            for k in range(1, 4):
                                nc.vector.scalar_tensor_tensor(
                                    out=tmp, in0=xin[:, k:k + NT],
                                    scalar=convw[:, et * 4 + k:et * 4 + k + 1],
                                    in1=tmp, op0=OP.mult, op1=OP.add)
                            nc.scalar.activation(out=xc[et], in_=tmp, func=AF.Silu,
                                                 bias=convb[:, et:et + 1], scale=1.0)
                            # mask is non-unit only on the warm-up columns
                            nc.vector.tensor_mul(
                                xc[et][:, 0:W], xc[et][:, 0:W], mask_sb)

                    for et, tc3 in order:
                        s2_chain(et, tc3)

                    # in_proj z-half + silu -> HBM scratch
                    for et in range(KC):
                        wt = ws.tile([128, KD, 128], F32R, tag="w1", bufs=6, name="wtz")
                        nc.sync.dma_start(
                            out=wt,
                            in_=w1z_h.ap()[:, et * 128:(et + 1) * 128]
                            .rearrange("(kt p) e -> p kt e", p=128).bitcast(F32R))
                        for tc3 in range(NTC):
                            ps = psmm.tile([128, TC], F32, tag="mm", name="psz")
                            for kt in range(KD):
                                nc.tensor.matmul(
                                    ps, wt[:, kt, :], xT[kt][tc3],
                                    start=(kt == 0), stop=(kt == KD - 1))
                            zroll = zrp.tile([128, TC], F32, tag="zr", name="zroll")
                            nc.scalar.activation(out=zroll, in_=ps, func=AF.Silu)
                            nc.sync.dma_start(
                                out=z_h.ap()[et, :, tc3 * TC:(tc3 + 1) * TC], in_=zroll)

                # ---- S4-S6: gate matmul, sigmoid, chunked scan, y*silu(z).
                # y is kept in SBUF as chunk tiles whose column slices feed
                # out_proj directly as stationary operands - no HBM roundtrip.
                with tc.tile_pool(name="yp", bufs=1) as yp:
                    ych = [[None] * NTC for _ in range(KC)]
                    with tc.tile_pool(name="gws", bufs=3) as gs, \
                         tc.tile_pool(name="ach", bufs=6) as ayp, \
                         tc.tile_pool(name="s6roll", bufs=4) as s6r, \
                         tc.tile_pool(name="psg", bufs=6, space="PSUM") as psg:

                        for et in range(KC):
                            gt = gs.tile([128, KC, 128], F32R, tag="gw", name="gt")
                            nc.gpsimd.dma_start(
                                out=gt,
                                in_=gw_h.ap()[:, et * 128:(et + 1) * 128]
                                .rearrange("(kt p) e -> p kt e", p=128).bitcast(F32R))
                            zls = []
                            for tc3 in range(NTC):
                                lo = max(tc3 * TC, W)
                                zl = s6r.tile([128, (tc3 + 1) * TC - lo], F32,
                                              tag="zl", name="zl")
                                nc.gpsimd.dma_start(
                                    out=zl, in_=z_h.ap()[et, :, lo:(tc3 + 1) * TC])
                                zls.append(zl)
                            prev_y = None
                            for tc3 in range(NTC):
                                a_t = ayp.tile([128, TC], F32R, tag="ach", name="ach")
                                ps = psg.tile([128, TC], F32, tag="mm", name="psgt")
                                for kt in range(KC):
                                    nc.tensor.matmul(
                                        ps, gt[:, kt, :],
                                        xc[kt][:, tc3 * TC:(tc3 + 1) * TC],
                                        start=(kt == 0), stop=(kt == KC - 1))
                                nc.scalar.activation(
                                    out=a_t, in_=ps,
                                    func=AF.Sigmoid, bias=gateb[:, et:et + 1], scale=1.0)
                                bt = s6r.tile([128, TC], F32, tag="bt", name="bt")
                                nc.vector.scalar_tensor_tensor(
                                    out=bt, in0=a_t, scalar=1.0,
                                    in1=xc[et][:, tc3 * TC:(tc3 + 1) * TC],
                                    op0=OP.subtract, op1=OP.mult)
                                y_t = yp.tile([128, TC], F32R, name=f"y{et}_{tc3}")
                                init = 0.0 if tc3 == 0 else prev_y[:, TC - 1:TC]
                                nc.vector.tensor_tensor_scan(
                                    out=y_t, data0=a_t, data1=bt, initial=init,
                                    op0=OP.mult, op1=OP.add)
                                ych[et][tc3] = y_t
                                prev_y = y_t
                            # -yg: multiply after the carry chain is complete
                            for tc3 in range(NTC):
                                lo = max(tc3 * TC, W) - tc3 * TC
                                nc.vector.tensor_mul(
                                    ych[et][tc3][:, lo:TC],
                                    ych[et][tc3][:, lo:TC], zls[tc3])

                    # ---- S7: out_proj + residual.  y chunk slices are the
                    # stationary operands; kt-major accumulation, two tb-half
                    # passes of 8 PSUM banks; opt streamed per (pass, kt). ----
                    NTB = CHUNK // 128

                    def yslice(kt, tb):
                        col = W + tb * 128          # absolute column in [0, NT)
                        tc3, off = col // TC, col % TC
                        return ych[kt][tc3][:, off:off + 128]

                    with tc.tile_pool(name="ops", bufs=3) as opp, \
                         tc.tile_pool(name="s7roll", bufs=4) as s7r, \
                         tc.tile_pool(name="s7res", bufs=5) as s7x, \
                         tc.tile_pool(name="psop", bufs=8, space="PSUM") as psop:
                        for half in range(2):
                            tbs = range(half * 4, half * 4 + 4)
                            xres = {}
                            for tb in tbs:
                                xres[tb] = s7x.tile([128, D], F32, tag="xres",
                                                    name=f"xres{tb}")
                                nc.sync.dma_start(
                                    out=xres[tb],
                                    in_=x_h.ap()[W + tb * 128:W + (tb + 1) * 128, :])
                            pss = {(tb, nb): psop.tile([128, 512], F32, tag="op",
                                                       name=f"pso{tb}_{nb}")
                                   for tb in tbs for nb in range(2)}
                            for kt in range(KC):
                                opt = opp.tile([128, D], F32R, tag="opw", name="opt")
                                nc.sync.dma_start(
                                    out=opt,
                                    in_=op_h.ap()[kt * 128:(kt + 1) * 128, :]
                                    .bitcast(F32R))
                                for tb in tbs:
                                    ys = yslice(kt, tb)
                                    for nb in range(2):
                                        nc.tensor.matmul(
                                            pss[(tb, nb)], ys,
                                            opt[:, nb * 512:(nb + 1) * 512],
                                            start=(kt == 0), stop=(kt == KC - 1))
                            for tb in tbs:
                                for nb in range(2):
                                    oh = s7r.tile([128, 512], F32, tag="oh", name="oh")
                                    nc.vector.tensor_sub(
                                        oh, xres[tb][:, nb * 512:(nb + 1) * 512],
                                        pss[(tb, nb)])
                                    nc.sync.dma_start(
                                        out=out_h.ap()[tb * 128:(tb + 1) * 128,
                                                       nb * 512:(nb + 1) * 512],
                                        in_=oh)

    nc.compile()
    return nc


def _prep_host(x, norm_w, norm_b, in_proj_w, conv_w, conv_b, gate_w, gate_b,
               out_proj_w):
    w1 = (in_proj_w * norm_w[None, :]).astype(np.float32)
    w1xT = np.ascontiguousarray(w1[:DI].T)           # [D, DI]
    w1zT = np.ascontiguousarray(w1[DI:].T)           # [D, DI]
    gwT = np.ascontiguousarray(gate_w.T)             # [DI, DI]
    opT = np.ascontiguousarray(out_proj_w.T)         # [DI, D]
    convw_r = np.ascontiguousarray(
        conv_w.reshape(KC, 128, 4).transpose(1, 0, 2).reshape(128, KC * 4))
    convb_r = np.ascontiguousarray(conv_b.reshape(KC, 128).T)
    gateb_r = np.ascontiguousarray(gate_b.reshape(KC, 128).T)
    normb_r = np.ascontiguousarray(norm_b.reshape(KD, 128).T)

    in_maps = []
    for core in range(8):
        b, j = core // 4, core % 4
        xs = np.zeros((NT, D), np.float32)
        start = j * CHUNK - W
        mask = np.ones((1, NT), np.float32)
        if j == 0:
            xs[W:] = x[b, 0:CHUNK]
            mask[0, :W] = 0.0
        else:
            xs[:] = x[b, start:start + NT]
        in_maps.append({
            "x": np.ascontiguousarray(xs), "w1x": w1xT, "w1z": w1zT,
            "gw": gwT, "opw": opT, "convw": convw_r, "convb": convb_r,
            "gateb": gateb_r, "normb": normb_r, "mask": mask,
        })
    return in_maps


def kernel(x, norm_w, norm_b, in_proj_w, conv_w, conv_b, gate_w, gate_b,
           out_proj_w, _trace=False, _collect=None):
    x = np.asarray(x, np.float32)
    if "nc" not in _cache:
        _cache["nc"] = _build()
    nc = _cache["nc"]
    in_maps = _prep_host(
        x, np.asarray(norm_w, np.float32), np.asarray(norm_b, np.float32),
        np.asarray(in_proj_w, np.float32), np.asarray(conv_w, np.float32),
        np.asarray(conv_b, np.float32), np.asarray(gate_w, np.float32),
        np.asarray(gate_b, np.float32), np.asarray(out_proj_w, np.float32))
    res = run_bass_kernel_spmd(nc, in_maps, core_ids=list(range(8)), trace=_trace)
    if _collect is not None:
        _collect.append(res)
    out = np.empty((B, L, D), np.float32)
    for core in range(8):
        b, j = core // 4, core % 4
        out[b, j * CHUNK:(j + 1) * CHUNK] = res.results[core]["out"]
    return out


# revision 25
# speedup vs baseline: 1.2321x; 1.0292x over previous
"""GatedLinearRecurrence Trainium2 kernel (8-core SPMD, Bass/Tile).

Strategy: shard (batch=2) x (4 sequence chunks of 1024 tokens) across 8 cores.
Each core processes 1152 tokens: a 128-token warm-up window (re-computed
redundantly; the recurrence decay e^{-~100} makes carry-in truncation error
~1e-24) followed by its 1024 "main" tokens.  No collectives needed.

Per-core pipeline (channels-on-partitions, tokens-on-free layout):
  LN(x) [t,d] -> PE-transpose -> x̂T [d,t] -> in_proj (f32r matmul)
  -> causal depthwise conv (4 shifted tensor_scalar ops) -> silu -> mask
  -> gate matmul -> sigmoid -> b=-(1-a)*xc -> tensor_tensor_scan (h=-h)
  -> y*silu(z) -> out_proj -> residual subtract -> out [t,dm].

The sign trick: scan data1 = (a-1)*x_conv = -b gives -h; -h*silu(z) = -yg;
out = x - matmul(-yg) = x + proj(yg).

Scheduling notes: x̂T is stored in 384-column chunk tiles so in_proj can
start before layernorm finishes; PSUM evacuations ride the Scalar engine
(Identity/Copy activations) to unload DVE; the scan is chunked (chained via
`initial=`) and interleaved into the gate loop so out_proj follows with no
PE gap; scan outputs reuse the a-tile pool slots (same tag) to fit SBUF.
"""
import sys

for p in ("/opt/trn_rl_repo", "/root/.axon_site/_ro/trn_rl_repo"):
    if p not in sys.path:
        sys.path.insert(0, p)

import numpy as np

import concourse.bass as bass
import concourse.bacc as bacc
import concourse.tile as tile
import concourse.mybir as mybir
from concourse.bass_utils import run_bass_kernel_spmd
from concourse.masks import make_identity

F32 = mybir.dt.float32
F32R = mybir.dt.float32r
AF = mybir.ActivationFunctionType
OP = mybir.AluOpType

B, L, D = 2, 4096, 1024
DI = 2048            # d_inner
NT = 1152            # tokens per core (128 warm-up + 1024 main)
W = 128              # warm-up tokens
CHUNK = 1024
NTT = NT // 128      # 9 token tiles
KD = D // 128        # 8 k-tiles over d_model
KC = DI // 128       # 16 k-tiles over d_inner
TC = 384             # matmul N chunk (3 per core)
NTC = NT // TC
EPS = 1e-5

_cache = {}


def _build():
    nc = bacc.Bacc(None, target_bir_lowering=False)

    x_h = nc.dram_tensor("x", [NT, D], F32, kind="ExternalInput")
    w1x_h = nc.dram_tensor("w1x", [D, DI], F32, kind="ExternalInput")
    w1z_h = nc.dram_tensor("w1z", [D, DI], F32, kind="ExternalInput")
    gw_h = nc.dram_tensor("gw", [DI, DI], F32, kind="ExternalInput")
    op_h = nc.dram_tensor("opw", [DI, D], F32, kind="ExternalInput")
    convw_h = nc.dram_tensor("convw", [128, KC * 4], F32, kind="ExternalInput")
    convb_h = nc.dram_tensor("convb", [128, KC], F32, kind="ExternalInput")
    gateb_h = nc.dram_tensor("gateb", [128, KC], F32, kind="ExternalInput")
    normb_h = nc.dram_tensor("normb", [128, KD], F32, kind="ExternalInput")
    mask_h = nc.dram_tensor("mask", [1, NT], F32, kind="ExternalInput")
    out_h = nc.dram_tensor("out", [CHUNK, D], F32, kind="ExternalOutput")
    z_h = nc.dram_tensor("z_scratch", [KC, 128, NT], F32, kind="Internal")
    yg_h = nc.dram_tensor("yg_scratch", [KC, 128, CHUNK], F32, kind="Internal")

    with tile.TileContext(nc) as tc:
        with tc.tile_pool(name="consts", bufs=1) as consts:

            ident = consts.tile([128, 128], F32, name="ident")
            make_identity(nc, ident)
            mask_sb = consts.tile([128, W], F32R, name="mask_sb")
            nc.gpsimd.dma_start(
                out=mask_sb,
                in_=bass.AP(tensor=mask_h, offset=0, ap=[[0, 128], [1, W]]).bitcast(F32R),
            )
            convw = consts.tile([128, KC * 4], F32, name="convw")
            nc.gpsimd.dma_start(out=convw, in_=convw_h.ap())
            convb = consts.tile([128, KC], F32, name="convb")
            nc.gpsimd.dma_start(out=convb, in_=convb_h.ap())
            gateb = consts.tile([128, KC], F32, name="gateb")
            nc.gpsimd.dma_start(out=gateb, in_=gateb_h.ap())
            normb = consts.tile([128, KD], F32, name="normb")
            nc.gpsimd.dma_start(out=normb, in_=normb_h.ap())
            eps_t = consts.tile([128, 1], F32, name="eps_t")
            nc.vector.memset(eps_t, EPS)

            with tc.tile_pool(name="xcp", bufs=1) as xcp:
                xc = [xcp.tile([128, NT], F32R, name=f"xct{e}") for e in range(KC)]

                # ---- S1-S3: LN, transpose, in_proj (x & z), conv, silu ----
                with tc.tile_pool(name="xT", bufs=1) as xTp, \
                     tc.tile_pool(name="s1roll", bufs=2) as s1r, \
                     tc.tile_pool(name="stat", bufs=4) as stp, \
                     tc.tile_pool(name="w1s", bufs=3) as ws, \
                     tc.tile_pool(name="zr", bufs=4) as zrp, \
                     tc.tile_pool(name="psmm", bufs=5, space="PSUM") as psmm, \
                     tc.tile_pool(name="pstr", bufs=2, space="PSUM") as pstr:

                    # x-hat-T chunk tiles [d-tile][t-chunk]: finer deps, so
                    # the first in_proj matmuls start after 3 LN iterations.
                    xT = [[xTp.tile([128, TC], F32R, name=f"xTt{d_}_{c_}")
                           for c_ in range(NTC)] for d_ in range(KD)]

                    for it in range(NTT):
                        tc3, col = it // 3, (it % 3) * 128
                        xt = s1r.tile([128, D], F32, tag="xt", bufs=3, name="xt")
                        nc.sync.dma_start(out=xt, in_=x_h.ap()[it * 128:(it + 1) * 128, :])
                        stats = stp.tile([128, 2, 6], F32, tag="stats", name="stats")
                        nc.vector.bn_stats(out=stats[:, 0, :], in_=xt[:, 0:512])
                        nc.vector.bn_stats(out=stats[:, 1, :], in_=xt[:, 512:1024])
                        mv = stp.tile([128, 2], F32, tag="mv", name="mv")
                        nc.vector.bn_aggr(out=mv, in_=stats)
                        rstd = stp.tile([128, 1], F32, tag="rstd", name="rstd")
                        nc.scalar.activation(out=rstd, in_=mv[:, 1:2], func=AF.Sqrt,
                                             bias=eps_t, scale=1.0)
                        nc.vector.reciprocal(out=rstd, in_=rstd)
                        nc.vector.tensor_scalar(out=xt, in0=xt, scalar1=mv[:, 0:1],
                                                scalar2=rstd, op0=OP.subtract, op1=OP.mult)
                        for d_ in range(KD):
                            pst = pstr.tile([128, 128], F32, tag="tr", name="pst")
                            nc.tensor.transpose(pst, xt[:, d_ * 128:(d_ + 1) * 128], ident)
                            # evac + norm_b on the Scalar engine
                            nc.scalar.activation(
                                out=xT[d_][tc3][:, col:col + 128], in_=pst,
                                func=AF.Identity, bias=normb[:, d_:d_ + 1], scale=1.0)

                    # in_proj x-half + conv + silu + warm-up mask.  The first
                    # 4 ets interleave their t-chunks so the PE never waits on
                    # layernorm chunks still in flight.
                    NW = 4
                    order = [(e, c) for c in range(NTC) for e in range(NW)]
                    order += [(e, c) for e in range(NW, KC) for c in range(NTC)]
                    wts, xins = {}, {}

                    def s2_chain(et, tc3):
                        if tc3 == 0:
                            wt = ws.tile([128, KD, 128], F32R, tag="w1",
                                         bufs=6, name=f"wt{et}")
                            nc.sync.dma_start(
                                out=wt,
                                in_=w1x_h.ap()[:, et * 128:(et + 1) * 128]
                                .rearrange("(kt p) e -> p kt e", p=128).bitcast(F32R))
                            wts[et] = wt
                            xin = s1r.tile([128, NT + 3], F32, tag="xin",
                                           bufs=NW + 2, name=f"xin{et}")
                            nc.vector.memset(xin[:, 0:3], 0.0)
                            xins[et] = xin
                        ps = psmm.tile([128, TC], F32, tag="mm", name="ps")
                        for kt in range(KD):
                            nc.tensor.matmul(
                                ps, wts[et][:, kt, :], xT[kt][tc3],
                                start=(kt == 0), stop=(kt == KD - 1))
                        nc.scalar.copy(
                            out=xins[et][:, 3 + tc3 * TC: 3 + (tc3 + 1) * TC], in_=ps)
                        if tc3 == NTC - 1:
                            xin = xins.pop(et)
                            tmp = s1r.tile([128, NT], F32, tag="ctmp", name="ctmp")
                            nc.vector.tensor_scalar_mul(
                                tmp, xin[:, 0:NT], convw[:, et * 4:et * 4 + 1])
                            for k in range(1, 4):
                                nc.vector.scalar_tensor_tensor(
                                    out=tmp, in0=xin[:, k:k + NT],
                                    scalar=convw[:, et * 4 + k:et * 4 + k + 1],
                                    in1=tmp, op0=OP.mult, op1=OP.add)
                            nc.scalar.activation(out=xc[et], in_=tmp, func=AF.Silu,
                                                 bias=convb[:, et:et + 1], scale=1.0)
                            # mask is non-unit only on the warm-up columns
                            nc.vector.tensor_mul(
                                xc[et][:, 0:W], xc[et][:, 0:W], mask_sb)

                    for et, tc3 in order:
                        s2_chain(et, tc3)

                    # in_proj z-half + silu -> HBM scratch
                    for et in range(KC):
                        wt = ws.tile([128, KD, 128], F32R, tag="w1", bufs=6, name="wtz")
                        nc.sync.dma_start(
                            out=wt,
                            in_=w1z_h.ap()[:, et * 128:(et + 1) * 128]
                            .rearrange("(kt p) e -> p kt e", p=128).bitcast(F32R))
                        for tc3 in range(NTC):
                            ps = psmm.tile([128, TC], F32, tag="mm", name="psz")
                            for kt in range(KD):
                                nc.tensor.matmul(
                                    ps, wt[:, kt, :], xT[kt][tc3],
                                    start=(kt == 0), stop=(kt == KD - 1))
                            zroll = zrp.tile([128, TC], F32, tag="zr", name="zroll")
                            nc.scalar.activation(out=zroll, in_=ps, func=AF.Silu)
                            nc.sync.dma_start(
                                out=z_h.ap()[et, :, tc3 * TC:(tc3 + 1) * TC], in_=zroll)

                # ---- S4-S6: gate matmul, sigmoid, chunked scan, y*silu(z).
                # y is kept in SBUF as chunk tiles whose column slices feed
                # out_proj directly as stationary operands - no HBM roundtrip.
                with tc.tile_pool(name="yp", bufs=1) as yp:
                    ych = [[None] * NTC for _ in range(KC)]
                    with tc.tile_pool(name="gws", bufs=3) as gs, \
                         tc.tile_pool(name="ach", bufs=6) as ayp, \
                         tc.tile_pool(name="s6roll", bufs=6) as s6r, \
                         tc.tile_pool(name="psg", bufs=6, space="PSUM") as psg:

                        for et in range(KC):
                            gt = gs.tile([128, KC, 128], F32R, tag="gw", name="gt")
                            nc.gpsimd.dma_start(
                                out=gt,
                                in_=gw_h.ap()[:, et * 128:(et + 1) * 128]
                                .rearrange("(kt p) e -> p kt e", p=128).bitcast(F32R))
                            zls = []
                            for tc3 in range(NTC):
                                lo = max(tc3 * TC, W)
                                zl = s6r.tile([128, (tc3 + 1) * TC - lo], F32,
                                              tag="zl", name="zl")
                                nc.gpsimd.dma_start(
                                    out=zl, in_=z_h.ap()[et, :, lo:(tc3 + 1) * TC])
                                zls.append(zl)
                            prev_y = None
                            for tc3 in range(NTC):
                                a_t = ayp.tile([128, TC], F32R, tag="ach", name="ach")
                                ps = psg.tile([128, TC], F32, tag="mm", name="psgt")
                                for kt in range(KC):
                                    nc.tensor.matmul(
                                        ps, gt[:, kt, :],
                                        xc[kt][:, tc3 * TC:(tc3 + 1) * TC],
                                        start=(kt == 0), stop=(kt == KC - 1))
                                nc.scalar.activation(
                                    out=a_t, in_=ps,
                                    func=AF.Sigmoid, bias=gateb[:, et:et + 1], scale=1.0)
                                bt = s6r.tile([128, TC], F32, tag="bt", name="bt")
                                nc.vector.scalar_tensor_tensor(
                                    out=bt, in0=a_t, scalar=1.0,
                                    in1=xc[et][:, tc3 * TC:(tc3 + 1) * TC],
                                    op0=OP.subtract, op1=OP.mult)
                                y_t = yp.tile([128, TC], F32R, name=f"y{et}_{tc3}")
                                init = 0.0 if tc3 == 0 else prev_y[:, TC - 1:TC]
                                nc.vector.tensor_tensor_scan(
                                    out=y_t, data0=a_t, data1=bt, initial=init,
                                    op0=OP.mult, op1=OP.add)
                                ych[et][tc3] = y_t
                                prev_y = y_t
                            # -yg: multiply after the carry chain is complete
                            for tc3 in range(NTC):
                                lo = max(tc3 * TC, W) - tc3 * TC
                                nc.vector.tensor_mul(
                                    ych[et][tc3][:, lo:TC],
                                    ych[et][tc3][:, lo:TC], zls[tc3])

                    # ---- S7: out_proj + residual.  y chunk slices are the
                    # stationary operands; kt-major accumulation, two tb-half
                    # passes of 8 PSUM banks; opt streamed per (pass, kt). ----
                    NTB = CHUNK // 128

                    def yslice(kt, tb):
                        col = W + tb * 128          # absolute column in [0, NT)
                        tc3, off = col // TC, col % TC
                        return ych[kt][tc3][:, off:off + 128]

                    with tc.tile_pool(name="ops", bufs=4) as opp, \
                         tc.tile_pool(name="s7roll", bufs=4) as s7r, \
                         tc.tile_pool(name="s7res", bufs=5) as s7x, \
                         tc.tile_pool(name="psop", bufs=8, space="PSUM") as psop:
                        for half in range(2):
                            tbs = range(half * 4, half * 4 + 4)
                            xres = {}
                            for tb in tbs:
                                xres[tb] = s7x.tile([128, D], F32, tag="xres",
                                                    name=f"xres{tb}")
                                nc.sync.dma_start(
                                    out=xres[tb],
                                    in_=x_h.ap()[W + tb * 128:W + (tb + 1) * 128, :])
                            pss = {(tb, nb): psop.tile([128, 512], F32, tag="op",
                                                       name=f"pso{tb}_{nb}")
                                   for tb in tbs for nb in range(2)}
                            for kt in range(KC):
                                opt = opp.tile([128, D], F32R, tag="opw", name="opt")
                                nc.sync.dma_start(
                                    out=opt,
                                    in_=op_h.ap()[kt * 128:(kt + 1) * 128, :]
                                    .bitcast(F32R))
                                for tb in tbs:
                                    ys = yslice(kt, tb)
                                    for nb in range(2):
                                        nc.tensor.matmul(
                                            pss[(tb, nb)], ys,
                                            opt[:, nb * 512:(nb + 1) * 512],
                                            start=(kt == 0), stop=(kt == KC - 1))
                            for tb in tbs:
                                for nb in range(2):
                                    oh = s7r.tile([128, 512], F32, tag="oh", name="oh")
                                    nc.vector.tensor_sub(
                                        oh, xres[tb][:, nb * 512:(nb + 1) * 512],
                                        pss[(tb, nb)])
                                    nc.sync.dma_start(
                                        out=out_h.ap()[tb * 128:(tb + 1) * 128,
                                                       nb * 512:(nb + 1) * 512],
                                        in_=oh)

    nc.compile()
    return nc


def _prep_host(x, norm_w, norm_b, in_proj_w, conv_w, conv_b, gate_w, gate_b,
               out_proj_w):
    w1 = (in_proj_w * norm_w[None, :]).astype(np.float32)
    w1xT = np.ascontiguousarray(w1[:DI].T)           # [D, DI]
    w1zT = np.ascontiguousarray(w1[DI:].T)           # [D, DI]
    gwT = np.ascontiguousarray(gate_w.T)             # [DI, DI]
    opT = np.ascontiguousarray(out_proj_w.T)         # [DI, D]
    convw_r = np.ascontiguousarray(
        conv_w.reshape(KC, 128, 4).transpose(1, 0, 2).reshape(128, KC * 4))
    convb_r = np.ascontiguousarray(conv_b.reshape(KC, 128).T)
    gateb_r = np.ascontiguousarray(gate_b.reshape(KC, 128).T)
    normb_r = np.ascontiguousarray(norm_b.reshape(KD, 128).T)

    in_maps = []
    for core in range(8):
        b, j = core // 4, core % 4
        xs = np.zeros((NT, D), np.float32)
        start = j * CHUNK - W
        mask = np.ones((1, NT), np.float32)
        if j == 0:
            xs[W:] = x[b, 0:CHUNK]
            mask[0, :W] = 0.0
        else:
            xs[:] = x[b, start:start + NT]
        in_maps.append({
            "x": np.ascontiguousarray(xs), "w1x": w1xT, "w1z": w1zT,
            "gw": gwT, "opw": opT, "convw": convw_r, "convb": convb_r,
            "gateb": gateb_r, "normb": normb_r, "mask": mask,
        })
    return in_maps


def kernel(x, norm_w, norm_b, in_proj_w, conv_w, conv_b, gate_w, gate_b,
           out_proj_w, _trace=False, _collect=None):
    x = np.asarray(x, np.float32)
    if "nc" not in _cache:
        _cache["nc"] = _build()
    nc = _cache["nc"]
    in_maps = _prep_host(
        x, np.asarray(norm_w, np.float32), np.asarray(norm_b, np.float32),
        np.asarray(in_proj_w, np.float32), np.asarray(conv_w, np.float32),
        np.asarray(conv_b, np.float32), np.asarray(gate_w, np.float32),
        np.asarray(gate_b, np.float32), np.asarray(out_proj_w, np.float32))
    res = run_bass_kernel_spmd(nc, in_maps, core_ids=list(range(8)), trace=_trace)
    if _collect is not None:
        _collect.append(res)
    out = np.empty((B, L, D), np.float32)
    for core in range(8):
        b, j = core // 4, core % 4
        out[b, j * CHUNK:(j + 1) * CHUNK] = res.results[core]["out"]
    return out
